# revision 16
# baseline (speedup 1.0000x reference)
"""Trainium2 Bass kernel for CausalAttentionSortNet bucket-scoring.

Math (see reference): only `k` feeds the output. For each merged batch*head
slice, the cumulative-average of k is sampled at bucket starts (every 128th
row), which reduces to per-chunk sums + a strictly-triangular prefix matmul.
The rest is tiny per-bucket sort projections and a 64x65 masked softmax.

Sharding: data-parallel over the merged (batch*heads)=32 axis across 8 cores,
4 slices per core, processed as 2 pairs of 2 slices; a pair fills the
128-partition dim as partition=(slice_in_pair, chunk), free=(pair, row, dim)
so every partition's k data is contiguous 32KB HBM runs (DMA saturates all
16 engines at ~350 GB/s).

`q` (half of all input bytes) is never read by the reference computation, so
it is not even transferred to the device.

DMA-instruction budget: the hardware exposes ~12 DMA completion semaphores,
so a DMA instruction >=12 positions later reuses an earlier one's semaphore
and its issue blocks until that user completes. All constants ship in two
early-completing DMAs and the bulk tiles are uniform, so every reuse target
is long done by the time its semaphore is recycled (a version that put a
slow small-packet constant DMA in the reuse chain stalled the bulk queue
12us). Chunk first-rows are not a separate DMA: they arrive inside each
pair's first bulk tile, whose fold targets the tile's upper half so row 0
survives for the F-term matmuls.

Per-chunk reduction: each pair's rows stream as sub-tiles of
(16x7, 8, 4, 4) rows. Mid-stream, SBUF port contention caps DVE at
~1.8ns/elem and GpSimd at ~2.6ns/elem (vs 1.04/2.0 idle), so each sub-tile
gets an INDEPENDENT halving-fold chain (contiguous tensor_adds down to one
row -> its own partial-sum slot) and the chains are statically balanced
across both engines; a long serial cascade on one engine trailed the
stream by 12us. The PE (otherwise idle) folds every partial into the
scaled-prefix via one matmul per sub-tile against the tril*scale constant,
accumulating in that pair's PSUM bank, opened by the F*diag(s) seed and
closed by the last sub-tile's matmul. Small sub-tiles stream last so the
post-stream tail is two ~0.3us fold chains plus the epilogue.
"""

from contextlib import ExitStack

import numpy as np

import concourse.bacc as bacc
import concourse.mybir as mybir
import concourse.tile as tile
from concourse import bass_utils

# Problem constants (hardcoded per contract; kernel.py must be self-contained).
B, HEADS, BUCKETS, DIM, DIM_SORT, T = 4, 8, 64, 64, 8, 8192
BH = B * HEADS            # 32 merged batch*head slices
NCORES = 8
BHC = BH // NCORES        # 4 slices per core
NPAIR = BHC // 2          # 2 pairs per core
CHUNK = T // BUCKETS      # 128 rows per bucket
NEG = -1.0e30             # softmax mask value (underflows exp to exactly 0)
FP = mybir.dt.float32

# packed-constant column offsets
NC128 = 128 * 3 + 72 * 2
NC64 = 4 * 104
NC104 = 2 * 128
NCALL = NC128 + NC104



TRACE = False  # set by test.py for profiling runs
TRACE_KWARGS = {}  # extra run_bass_kernel_spmd kwargs for profiling runs
LAST_RESULTS = None  # BassKernelResults of the most recent run

_PROG_CACHE = {}


# sub-tile plan, stream order: name, rows, merged-pair?, engine.
# Pair-MERGED tiles hold both pairs in one tile (one fold chain covers both
# -> ~30% less engine time); per-pair SPLIT tiles give the scheduler finer
# arrival granularity for the DVE. GpSimd (half DVE's effective rate) gets
# one big early chain plus small late ones; everything else on DVE.
# Rows per pair: 32(A)+32(B)+32(C)+16(D)+8(E)+4(F)+4(G) = 128.
TILE_PLAN = (
    # (tag, rows, row0, merged, engine, early-queue)
    ("A1", 32, 0, 1, "v", True),    # pair 1 A, split
    ("Bm", 32, 32, None, "g", False),  # merged
    ("A0", 32, 0, 0, "v", False),
    ("C1", 32, 64, 1, "v", False),
    ("C0", 32, 64, 0, "v", False),
    ("Dm", 16, 96, None, "v", False),
    ("Em", 8, 112, None, "g", False),
    ("Fm", 4, 120, None, "g", False),
    ("Gm", 4, 124, None, "v", False),
)


def _build_program(t_seq=T, enable_asserts=False, debug_taps=False):
    chunk = t_seq // BUCKETS
    assert chunk == CHUNK, "sub-tile schedule is tuned for chunk=128"
    nsub = len(TILE_PLAN)

    nc = bacc.Bacc(
        "TRN2",
        target_bir_lowering=False,
        debug=False,
        enable_asserts=enable_asserts,
        num_devices=NCORES,
    )

    def din(name, shape):
        return nc.dram_tensor(name, shape, FP, kind="ExternalInput").ap()

    kin = din("kin", (BHC, t_seq, DIM))
    # packed constants, two DMAs:
    # cpack cols 0:656     [lmat_s | idents | ident | am68 | mm68]
    # cpack cols 644:900   c104 = per pair (104, 128) cq/ck blocks (rows 104:128 zero)
    # c64 (64, 416)        [wqk_pt_p0 | wqk_pt_p1 | wqk_ft_p0 | wqk_ft_p1]
    cpack = din("cpack", (128, NCALL))
    c64 = din("c64", (64, NC64))
    rout = nc.dram_tensor(
        "rout", (BHC, BUCKETS, BUCKETS + 1), FP, kind="ExternalOutput"
    ).ap()
    taps = {}
    if debug_taps:
        taps["pt"] = nc.dram_tensor("tap_pt", (128, 128), FP, kind="ExternalOutput").ap()
        taps["ft"] = nc.dram_tensor("tap_ft", (128, 128), FP, kind="ExternalOutput").ap()
        taps["par"] = nc.dram_tensor("tap_par", (128, 128), FP, kind="ExternalOutput").ap()

    X = mybir.AxisListType.X
    Exp = mybir.ActivationFunctionType.Exp
    MULT = mybir.AluOpType.mult

    with tile.TileContext(nc) as tc:
        with ExitStack() as ctx:
            singles = ctx.enter_context(tc.tile_pool(name="singles", bufs=1))
            kpools = [
                ctx.enter_context(tc.tile_pool(name=f"kpool{s}", bufs=1))
                for s in range(nsub)
            ]
            parp = ctx.enter_context(tc.tile_pool(name="parp", bufs=nsub))
            small = ctx.enter_context(tc.tile_pool(name="small", bufs=2))
            pp = ctx.enter_context(tc.tile_pool(name="pp", bufs=1, space="PSUM"))

            cp_sb = singles.tile([128, NCALL], FP, tag="cpack")
            nc.scalar.dma_start(cp_sb[:], cpack)
            c64_sb = singles.tile([64, NC64], FP, tag="c64")
            nc.scalar.dma_start(c64_sb[:], c64)

            # ---- bulk k sub-tile DMAs. The first tile issues from the
            # GpSimd (SWDGE) queue, which exits the framework preamble
            # ~0.6us before the sync queue, so its packets start earlier;
            # the rest stream from the sync queue (rows*256B runs).
            ksrcs = [
                kin[2 * p : 2 * p + 2].rearrange("b (c r) d -> (b c) r d", r=chunk)
                for p in range(NPAIR)
            ]
            kmerged = kin.rearrange(
                "(p b) (c r) d -> (b c) p r d", p=NPAIR, r=chunk
            )
            kts = {}
            for s, (tag, rs, r0, merged_p, _eng, early) in enumerate(TILE_PLAN):
                q = nc.gpsimd if early else nc.sync
                if merged_p is None:
                    kt = kpools[s].tile(
                        [128, NPAIR, rs, DIM], FP, tag=f"kt{tag}", name=f"kt{tag}"
                    )
                    q.dma_start(kt[:], kmerged[:, :, r0 : r0 + rs, :])
                else:
                    kt = kpools[s].tile(
                        [128, rs, DIM], FP, tag=f"kt{tag}", name=f"kt{tag}"
                    )
                    q.dma_start(kt[:], ksrcs[merged_p][:, r0 : r0 + rs, :])
                kts[tag] = kt

            lmat_s = cp_sb[:, 0:128]
            idents = cp_sb[:, 128:256]
            ident = cp_sb[:, 256:384]
            # am68 (cols 384:520) is consumed directly by the R-group matmul
            mm68 = cp_sb[:, 520:656].rearrange("q (p j) -> q p j", p=2)

            # ---- PSUM groups, one bank per (pair, tensor): FT_p is F
            # transposed; PT_p is opened by the F*diag(s) seed and closed by
            # that pair's chunk-sum prefix matmul. F = row 0 of the first
            # bulk tile. Separate banks let pair 0's whole epilogue run while
            # pair 1 is still streaming.
            # full-partition PSUM tiles: a 64-partition tile can be packed
            # at partition offset 64 of another tile's bank, and its group's
            # deferred-zero bookkeeping then poisons that tile's rows 64:128
            PT_ps = [
                pp.tile([128, 128], FP, tag=f"PT{p}", name=f"PT_ps{p}")
                for p in range(NPAIR)
            ]
            FT_ps = [
                pp.tile([128, 128], FP, tag=f"FT{p}", name=f"FT_ps{p}")
                for p in range(NPAIR)
            ]
            for p in range(NPAIR):
                nc.tensor.matmul(
                    FT_ps[p][0:64, :],
                    lhsT=kts[f"A{p}"][:, 0, :],
                    rhs=ident,
                    start=True,
                    stop=True,
                )
                nc.tensor.matmul(
                    PT_ps[p][0:64, :],
                    lhsT=kts[f"A{p}"][:, 0, :],
                    rhs=idents,
                    start=True,
                    stop=False,
                )

            # ---- per-sub-tile fold chains + per-sub-tile prefix matmuls.
            # Each chain folds its tile down to one row (first fold targets
            # the upper half so row 0, the F term, survives in tile 0) into
            # its own partial-sum slot; the PE folds every partial into the
            # pair's scaled-prefix PSUM bank as it appears.
            def fold_chain(e, view, viewr, rs, par_dst):
                # view(a, b) slices rows [a, b); viewr(r) indexes one row;
                # row 0 is preserved (first fold targets the upper half)
                h = rs // 2
                e.tensor_add(view(h, rs), view(h, rs), view(0, h))
                lo, xr = h, h
                while xr > 2:
                    hh = xr // 2
                    e.tensor_add(
                        view(lo, lo + hh), view(lo, lo + hh),
                        view(lo + hh, lo + xr),
                    )
                    xr = hh
                e.tensor_add(par_dst, viewr(lo), viewr(lo + 1))

            pars = {}
            for s, (tag, rs, r0, merged_p, eng, _early) in enumerate(TILE_PLAN):
                t = kts[tag]
                e = nc.gpsimd if eng == "g" else nc.vector
                par = parp.tile(
                    [128, NPAIR, DIM], FP, tag=f"par{tag}", name=f"par{tag}"
                )
                pars[tag] = par
                last = s == nsub - 1
                if merged_p is None:
                    fold_chain(
                        e,
                        lambda a, b, t=t: t[:, :, a:b, :],
                        lambda r, t=t: t[:, :, r, :],
                        rs,
                        par[:, :, :],
                    )
                    for p in (1, 0):
                        nc.tensor.matmul(
                            PT_ps[p][0:64, :],
                            lhsT=par[:, p, :],
                            rhs=lmat_s,
                            start=False,
                            stop=last,
                        )
                else:
                    p = merged_p
                    fold_chain(
                        e,
                        lambda a, b, t=t: t[:, a:b, :],
                        lambda r, t=t: t[:, r, :],
                        rs,
                        par[:, p, :],
                    )
                    nc.tensor.matmul(
                        PT_ps[p][0:64, :],
                        lhsT=par[:, p, :],
                        rhs=lmat_s,
                        start=False,
                        stop=last,
                    )

            # ---- sort projections (per pair), batched softmax (both pairs)
            PT_sb = [
                small.tile([64, 128], FP, tag=f"PTs{p}", name=f"PT_sb{p}")
                for p in range(NPAIR)
            ]
            FT_sb = [
                small.tile([64, 128], FP, tag=f"FTs{p}", name=f"FT_sb{p}")
                for p in range(NPAIR)
            ]
            for p in range(NPAIR):
                nc.scalar.copy(FT_sb[p][:], FT_ps[p][0:64, :])
                nc.scalar.copy(PT_sb[p][:], PT_ps[p][0:64, :])
            if debug_taps:
                for p in range(NPAIR):
                    nc.sync.dma_start(taps["pt"][64 * p : 64 * p + 64], PT_sb[p][:])
                    nc.sync.dma_start(taps["ft"][64 * p : 64 * p + 64], FT_sb[p][:])


            # SKQ rows: 0:40 sort-q blocks (b0 at 0:8, b1 at 32:40),
            #           64:104 sort-k blocks (b0 at 64:72, b1 at 96:104);
            # one PSUM bank per pair: each holds a long-open accumulation group
            # opened by the constant-term matmul (ready at kernel start) and
            # closed by the PT-part matmul (the only one on the critical tail)
            C104O = NC128
            SQs = []
            RKs = []
            for p in range(NPAIR):
                sk_ps_t = pp.tile([128, 128], FP, tag=f"SKQ{p}", name=f"skq{p}")
                sk_ps = sk_ps_t[0:104, :]
                nc.tensor.matmul(
                    sk_ps,
                    lhsT=ident[0:104, 0:104],
                    rhs=cp_sb[0:104, C104O + 128 * p : C104O + 128 * p + 128],
                    start=True,
                    stop=False,
                    skip_group_check=True,
                )
                nc.tensor.matmul(
                    sk_ps,
                    lhsT=c64_sb[:, 208 + 104 * p : 312 + 104 * p],
                    rhs=FT_sb[p][:],
                    start=False,
                    stop=False,
                    skip_group_check=True,
                )
                nc.tensor.matmul(
                    sk_ps,
                    lhsT=c64_sb[:, 104 * p : 104 * p + 104],
                    rhs=PT_sb[p][:],
                    start=False,
                    stop=True,
                    skip_group_check=True,
                )
                sq_sb = small.tile([40, 128], FP, tag=f"SQ{p}")
                nc.scalar.copy(sq_sb[:], sk_ps[0:40, :])
                rk_sb = small.tile([40, 128], FP, tag=f"RK{p}")
                nc.vector.tensor_copy(rk_sb[:], sk_ps[64:104, :])
                SQs.append(sq_sb)
                RKs.append(rk_sb)

            # R group, 65 logit columns per pair: opened early by an
            # identity-weighted matmul that seeds the bank with the additive
            # causal mask (whose column 0 is the pad-row's constant zero
            # logit); the four sq.sk matmuls then accumulate into columns
            # 1:65 of their quadrants, so the masked logits sit in PSUM with
            # no extra elementwise pass and no separate zero-column handling
            # R groups, one PSUM bank per pair, 72 logit columns: col 0
            # pad, col 1 the pad-row's constant zero logit (both written
            # only by the mask seed), cols 2:66 the sq.sk logits, 66:72
            # pad. The 72 width keeps the partition-64 quadrant writes
            # inside the seed-cleared PSUM zero-region window (64 *
            # width-bytes must be 0 mod 2048) and 8-byte aligned. Separate
            # banks let pair 0's entire softmax run while pair 1 is still
            # streaming; only pair 1's path sits on the tail.
            am72 = cp_sb[:, 384:456]
            mm72 = cp_sb[:, 456:528]
            for p in range(NPAIR):
                R_ps = pp.tile([128, 72], FP, tag=f"R{p}", name=f"R_ps{p}")
                nc.tensor.matmul(
                    R_ps[:],
                    lhsT=ident,
                    rhs=am72,
                    start=True,
                    stop=False,
                    skip_group_check=True,
                )
                nc.tensor.matmul(
                    R_ps[0:64, 2:66],
                    lhsT=SQs[p][0:8, 0:64],
                    rhs=RKs[p][0:8, 0:64],
                    start=False,
                    stop=False,
                    skip_group_check=True,
                )
                nc.tensor.matmul(
                    R_ps[64:128, 2:66],
                    lhsT=SQs[p][32:40, 64:128],
                    rhs=RKs[p][32:40, 64:128],
                    start=False,
                    stop=True,
                    skip_group_check=True,
                )

                # masked softmax over the 65 logits (cols 1:66); pads give
                # exp(NEG) = 0 so they never affect max/sum
                mx = small.tile([128, 1], FP, tag=f"mx{p}", name=f"mx{p}")
                nc.vector.reduce_max(mx[:], R_ps[:], axis=X)
                negm = small.tile([128, 1], FP, tag=f"ngm{p}", name=f"ngm{p}")
                nc.vector.tensor_scalar(
                    negm[:], mx[:], 0.0, -1.0,
                    op0=mybir.AluOpType.max, op1=MULT,
                )
                e_sb = small.tile([128, 72], FP, tag=f"e{p}", name=f"e{p}")
                nc.scalar.activation(
                    e_sb[:], R_ps[:], Exp, bias=negm[:], scale=1.0
                )
                s1 = small.tile([128, 1], FP, tag=f"s1{p}", name=f"s1{p}")
                nc.vector.reduce_sum(s1[:], e_sb[:], axis=X)
                rin = small.tile([128, 1], FP, tag=f"rin{p}", name=f"rin{p}")
                nc.vector.reciprocal(rin[:], s1[:])
                outt = small.tile([128, 72], FP, tag=f"ot{p}", name=f"ot{p}")
                # outt = (e * 1/den) * tril-mask, fused
                nc.vector.scalar_tensor_tensor(
                    outt[:], e_sb[:], rin[:], mm72,
                    op0=MULT,
                    op1=MULT,
                )
                # per-pair output DMA on its own queue so pair 0's issues
                # mid-stream and only pair 1's sits on the tail
                oq = nc.scalar if p == 0 else nc.sync
                oq.dma_start(
                    rout.rearrange("(p b) i c -> (b i) p c", p=2)[:, p, :],
                    outt[:, 1:66],
                )

    nc.compile()
    return nc


def _get_program(t_seq=T, enable_asserts=False):
    key = (t_seq, enable_asserts)
    if key not in _PROG_CACHE:
        _PROG_CACHE[key] = _build_program(t_seq, enable_asserts=enable_asserts)
    return _PROG_CACHE[key]


def _host_constants(core, q_pos_emb, k_pos_emb, Wsq, Wsk, chunk=CHUNK):
    """Single packed per-core constant tensor."""
    f32 = np.float32
    j = np.arange(64, dtype=np.float64)
    s = (1.0 / (chunk * j + 1.0)).astype(f32)  # per-bucket cumavg scale

    tri = np.triu(np.ones((64, 64), f32), k=1)  # [c, j] = 1 iff c < j
    tri_s = tri * s[None, :]
    lmat_s = np.zeros((128, 128), f32)
    lmat_s[0:64, 0:64] = tri_s
    lmat_s[64:128, 64:128] = tri_s
    idents = np.zeros((128, 128), f32)
    idents[np.arange(128), np.arange(128)] = np.concatenate([s, s])
    ident = np.eye(128, dtype=f32)

    rows = np.arange(64)[:, None]
    jj = np.arange(65)[None, :]
    # 72-wide block (same for both pairs): cols 0/66:72 pad (NEG / 0), col
    # 1+j for logit j: valid iff j <= i (j=0 = pad row's constant zero
    # logit), output keeps j < i
    am = np.full((64, 72), NEG, f32)
    am[:, 1:66] = np.where(jj <= rows, 0.0, NEG)
    mm = np.zeros((64, 72), f32)
    mm[:, 1:66] = (jj < rows).astype(f32)
    am72 = np.concatenate([am, am], axis=0)
    mm72 = np.concatenate([mm, mm], axis=0)

    c128 = np.concatenate([lmat_s, idents, ident, am72, mm72], axis=1)

    wq_pt = np.zeros((2, 64, 104), f32)   # [pair][d][sq 0:40 | sk 64:104]
    wq_ft = np.zeros((2, 64, 104), f32)
    cblk = np.zeros((2, 104, 128), f32)   # [pair][skq-row][(b, j)]
    for p in range(NPAIR):
        for b in range(2):
            bh = core * BHC + 2 * p + b
            h = bh % HEADS
            r0 = 32 * b
            wq_pt[p, :, r0 : r0 + 8] = Wsq[0, h, 0:64, :]
            wq_pt[p, :, 64 + r0 : 64 + r0 + 8] = Wsk[0, h, 0:64, :]
            wq_ft[p, :, r0 : r0 + 8] = Wsq[0, h, 64:128, :]
            wq_ft[p, :, 64 + r0 : 64 + r0 + 8] = Wsk[0, h, 64:128, :]
            cq = q_pos_emb[0, h] @ Wsq[0, h, 128:192, :]  # (64, 8)
            ck = k_pos_emb[0, h] @ Wsk[0, h, 128:192, :]
            cblk[p, r0 : r0 + 8, 64 * b : 64 * b + 64] = cq.T
            cblk[p, 64 + r0 : 64 + r0 + 8, 64 * b : 64 * b + 64] = ck.T

    c64 = np.concatenate([wq_pt[0], wq_pt[1], wq_ft[0], wq_ft[1]], axis=1)
    c104 = np.concatenate([cblk[0], cblk[1]], axis=1)
    c104 = np.concatenate([c104, np.zeros((24, NC104), f32)], axis=0)
    cpack = np.concatenate([c128, c104], axis=1)
    assert cpack.shape == (128, NCALL), cpack.shape
    assert c64.shape == (64, NC64), c64.shape
    return {"cpack": cpack, "c64": c64}


def _run(k, q_pos_emb, k_pos_emb, Wsq, Wsk, trace=False, t_seq=T):
    nc = _get_program(t_seq)
    in_maps = []
    for core in range(NCORES):
        cm = _host_constants(
            core, q_pos_emb, k_pos_emb, Wsq, Wsk, chunk=t_seq // BUCKETS
        )
        cm["kin"] = np.ascontiguousarray(k[core * BHC : (core + 1) * BHC])
        in_maps.append(cm)
    res = bass_utils.run_bass_kernel_spmd(
        nc,
        in_maps,
        core_ids=list(range(NCORES)),
        trace=trace,
        **(TRACE_KWARGS if trace else {}),
    )
    global LAST_RESULTS
    LAST_RESULTS = res
    out = np.concatenate([r["rout"] for r in res.results], axis=0)
    return out, res


def kernel(**inputs):
    k = np.asarray(inputs["k"], np.float32)
    q_pos_emb = np.asarray(inputs["q_pos_emb"], np.float32)
    k_pos_emb = np.asarray(inputs["k_pos_emb"], np.float32)
    Wsq = np.asarray(inputs["Wsq"], np.float32)
    Wsk = np.asarray(inputs["Wsk"], np.float32)
    out, _ = _run(k, q_pos_emb, k_pos_emb, Wsq, Wsk, trace=TRACE)
    return out


# revision 17
# speedup vs baseline: 1.0511x; 1.0511x over previous
"""Trainium2 Bass kernel for CausalAttentionSortNet bucket-scoring.

Math (see reference): only `k` feeds the output. For each merged batch*head
slice, the cumulative-average of k is sampled at bucket starts (every 128th
row), which reduces to per-chunk sums + a strictly-triangular prefix matmul.
The rest is tiny per-bucket sort projections and a 64x65 masked softmax.

Sharding: data-parallel over the merged (batch*heads)=32 axis across 8 cores,
4 slices per core, processed as 2 pairs of 2 slices; a pair fills the
128-partition dim as partition=(slice_in_pair, chunk), free=(pair, row, dim)
so every partition's k data is contiguous 32KB HBM runs (DMA saturates all
16 engines at ~350 GB/s).

`q` (half of all input bytes) is never read by the reference computation, so
it is not even transferred to the device.

DMA-instruction budget: the hardware exposes ~12 DMA completion semaphores,
so a DMA instruction >=12 positions later reuses an earlier one's semaphore
and its issue blocks until that user completes. All constants ship in two
early-completing DMAs and the bulk tiles are uniform, so every reuse target
is long done by the time its semaphore is recycled (a version that put a
slow small-packet constant DMA in the reuse chain stalled the bulk queue
12us). Chunk first-rows are not a separate DMA: they arrive inside each
pair's first bulk tile, whose fold targets the tile's upper half so row 0
survives for the F-term matmuls.

Per-chunk reduction: each pair's rows stream as sub-tiles of
(16x7, 8, 4, 4) rows. Mid-stream, SBUF port contention caps DVE at
~1.8ns/elem and GpSimd at ~2.6ns/elem (vs 1.04/2.0 idle), so each sub-tile
gets an INDEPENDENT halving-fold chain (contiguous tensor_adds down to one
row -> its own partial-sum slot) and the chains are statically balanced
across both engines; a long serial cascade on one engine trailed the
stream by 12us. The PE (otherwise idle) folds every partial into the
scaled-prefix via one matmul per sub-tile against the tril*scale constant,
accumulating in that pair's PSUM bank, opened by the F*diag(s) seed and
closed by the last sub-tile's matmul. Small sub-tiles stream last so the
post-stream tail is two ~0.3us fold chains plus the epilogue.
"""

from contextlib import ExitStack

import numpy as np

import concourse.bacc as bacc
import concourse.mybir as mybir
import concourse.tile as tile
from concourse import bass_utils

# Problem constants (hardcoded per contract; kernel.py must be self-contained).
B, HEADS, BUCKETS, DIM, DIM_SORT, T = 4, 8, 64, 64, 8, 8192
BH = B * HEADS            # 32 merged batch*head slices
NCORES = 8
BHC = BH // NCORES        # 4 slices per core
NPAIR = BHC // 2          # 2 pairs per core
CHUNK = T // BUCKETS      # 128 rows per bucket
NEG = -1.0e30             # softmax mask value (underflows exp to exactly 0)
FP = mybir.dt.float32

# packed-constant column offsets
NC128 = 128 * 3 + 72 * 2
NC64 = 4 * 104
NC104 = 2 * 128
NCALL = NC128 + NC104



TRACE = False  # set by test.py for profiling runs
TRACE_KWARGS = {}  # extra run_bass_kernel_spmd kwargs for profiling runs
LAST_RESULTS = None  # BassKernelResults of the most recent run

_PROG_CACHE = {}


# sub-tile plan, stream order: name, rows, merged-pair?, engine.
# Pair-MERGED tiles hold both pairs in one tile (one fold chain covers both
# -> ~30% less engine time); per-pair SPLIT tiles give the scheduler finer
# arrival granularity for the DVE. GpSimd (half DVE's effective rate) gets
# one big early chain plus small late ones; everything else on DVE.
# Rows per pair: 32(A)+32(B)+32(C)+16(D)+8(E)+8(F) = 128.
TILE_PLAN = (
    # (tag, rows, row0, merged, engine)
    ("A1", 32, 0, 1, "v"),      # pair 1 A, split
    ("Bm", 32, 32, None, "g"),  # merged: one chain covers both pairs
    ("C1", 32, 64, 1, "v"),
    ("A0", 32, 0, 0, "v"),
    ("C0", 32, 64, 0, "v"),
    ("D1", 16, 96, 1, "g"),
    ("D0", 16, 96, 0, "v"),
    ("Em", 8, 112, None, "g"),
    ("Fm", 8, 120, None, "v"),
)


def _build_program(t_seq=T, enable_asserts=False, debug_taps=False):
    chunk = t_seq // BUCKETS
    assert chunk == CHUNK, "sub-tile schedule is tuned for chunk=128"
    nsub = len(TILE_PLAN)

    nc = bacc.Bacc(
        "TRN2",
        target_bir_lowering=False,
        debug=False,
        enable_asserts=enable_asserts,
        num_devices=NCORES,
    )

    def din(name, shape):
        return nc.dram_tensor(name, shape, FP, kind="ExternalInput").ap()

    kin = din("kin", (BHC, t_seq, DIM))
    # packed constants, two DMAs:
    # cpack cols 0:656     [lmat_s | idents | ident | am68 | mm68]
    # cpack cols 644:900   c104 = per pair (104, 128) cq/ck blocks (rows 104:128 zero)
    # c64 (64, 416)        [wqk_pt_p0 | wqk_pt_p1 | wqk_ft_p0 | wqk_ft_p1]
    cpack = din("cpack", (128, NCALL))
    c64 = din("c64", (64, NC64))
    rout = nc.dram_tensor(
        "rout", (BHC, BUCKETS, BUCKETS + 1), FP, kind="ExternalOutput"
    ).ap()
    taps = {}
    if debug_taps:
        taps["pt"] = nc.dram_tensor("tap_pt", (128, 128), FP, kind="ExternalOutput").ap()
        taps["ft"] = nc.dram_tensor("tap_ft", (128, 128), FP, kind="ExternalOutput").ap()
        taps["par"] = nc.dram_tensor("tap_par", (128, 128), FP, kind="ExternalOutput").ap()

    X = mybir.AxisListType.X
    Exp = mybir.ActivationFunctionType.Exp
    MULT = mybir.AluOpType.mult

    with tile.TileContext(nc) as tc:
        with ExitStack() as ctx:
            singles = ctx.enter_context(tc.tile_pool(name="singles", bufs=1))
            kpools = [
                ctx.enter_context(tc.tile_pool(name=f"kpool{s}", bufs=1))
                for s in range(nsub)
            ]
            parp = ctx.enter_context(tc.tile_pool(name="parp", bufs=nsub))
            small = ctx.enter_context(tc.tile_pool(name="small", bufs=2))
            pp = ctx.enter_context(tc.tile_pool(name="pp", bufs=1, space="PSUM"))

            cp_sb = singles.tile([128, NCALL], FP, tag="cpack")
            nc.scalar.dma_start(cp_sb[:], cpack)
            c64_sb = singles.tile([64, NC64], FP, tag="c64")
            nc.scalar.dma_start(c64_sb[:], c64)

            # ---- bulk k sub-tile DMAs, all on the sync queue (a
            # side-queue experiment starved the DVE: its packets trailed
            # the sync stream by 10us); contiguous rows*256B runs.
            ksrcs = [
                kin[2 * p : 2 * p + 2].rearrange("b (c r) d -> (b c) r d", r=chunk)
                for p in range(NPAIR)
            ]
            kmerged = kin.rearrange(
                "(p b) (c r) d -> (b c) p r d", p=NPAIR, r=chunk
            )
            kts = {}
            for s, (tag, rs, r0, merged_p, _eng) in enumerate(TILE_PLAN):
                q = nc.sync
                if merged_p is None:
                    kt = kpools[s].tile(
                        [128, NPAIR, rs, DIM], FP, tag=f"kt{tag}", name=f"kt{tag}"
                    )
                    q.dma_start(kt[:], kmerged[:, :, r0 : r0 + rs, :])
                else:
                    kt = kpools[s].tile(
                        [128, rs, DIM], FP, tag=f"kt{tag}", name=f"kt{tag}"
                    )
                    q.dma_start(kt[:], ksrcs[merged_p][:, r0 : r0 + rs, :])
                kts[tag] = kt

            lmat_s = cp_sb[:, 0:128]
            idents = cp_sb[:, 128:256]
            ident = cp_sb[:, 256:384]
            # am68 (cols 384:520) is consumed directly by the R-group matmul
            mm68 = cp_sb[:, 520:656].rearrange("q (p j) -> q p j", p=2)

            # ---- PSUM groups, one bank per (pair, tensor): FT_p is F
            # transposed; PT_p is opened by the F*diag(s) seed and closed by
            # that pair's chunk-sum prefix matmul. F = row 0 of the first
            # bulk tile. Separate banks let pair 0's whole epilogue run while
            # pair 1 is still streaming.
            # full-partition PSUM tiles: a 64-partition tile can be packed
            # at partition offset 64 of another tile's bank, and its group's
            # deferred-zero bookkeeping then poisons that tile's rows 64:128
            PT_ps = [
                pp.tile([128, 128], FP, tag=f"PT{p}", name=f"PT_ps{p}")
                for p in range(NPAIR)
            ]
            FT_ps = [
                pp.tile([128, 128], FP, tag=f"FT{p}", name=f"FT_ps{p}")
                for p in range(NPAIR)
            ]
            for p in range(NPAIR):
                nc.tensor.matmul(
                    FT_ps[p][0:64, :],
                    lhsT=kts[f"A{p}"][:, 0, :],
                    rhs=ident,
                    start=True,
                    stop=True,
                )
                nc.tensor.matmul(
                    PT_ps[p][0:64, :],
                    lhsT=kts[f"A{p}"][:, 0, :],
                    rhs=idents,
                    start=True,
                    stop=False,
                )

            # ---- per-sub-tile fold chains + per-sub-tile prefix matmuls.
            # Each chain folds its tile down to one row (first fold targets
            # the upper half so row 0, the F term, survives in tile 0) into
            # its own partial-sum slot; the PE folds every partial into the
            # pair's scaled-prefix PSUM bank as it appears.
            def fold_chain(e, view, viewr, rs, par_dst):
                # view(a, b) slices rows [a, b); viewr(r) indexes one row;
                # row 0 is preserved (first fold targets the upper half)
                h = rs // 2
                e.tensor_add(view(h, rs), view(h, rs), view(0, h))
                lo, xr = h, h
                while xr > 2:
                    hh = xr // 2
                    e.tensor_add(
                        view(lo, lo + hh), view(lo, lo + hh),
                        view(lo + hh, lo + xr),
                    )
                    xr = hh
                e.tensor_add(par_dst, viewr(lo), viewr(lo + 1))

            pars = {}
            for s, (tag, rs, r0, merged_p, eng) in enumerate(TILE_PLAN):
                t = kts[tag]
                e = nc.gpsimd if eng == "g" else nc.vector
                par = parp.tile(
                    [128, NPAIR, DIM], FP, tag=f"par{tag}", name=f"par{tag}"
                )
                pars[tag] = par
                last = s == nsub - 1
                if merged_p is None:
                    fold_chain(
                        e,
                        lambda a, b, t=t: t[:, :, a:b, :],
                        lambda r, t=t: t[:, :, r, :],
                        rs,
                        par[:, :, :],
                    )
                    for p in (1, 0):
                        nc.tensor.matmul(
                            PT_ps[p][0:64, :],
                            lhsT=par[:, p, :],
                            rhs=lmat_s,
                            start=False,
                            stop=last,
                        )
                else:
                    p = merged_p
                    fold_chain(
                        e,
                        lambda a, b, t=t: t[:, a:b, :],
                        lambda r, t=t: t[:, r, :],
                        rs,
                        par[:, p, :],
                    )
                    nc.tensor.matmul(
                        PT_ps[p][0:64, :],
                        lhsT=par[:, p, :],
                        rhs=lmat_s,
                        start=False,
                        stop=last,
                    )

            # ---- sort projections (per pair), batched softmax (both pairs)
            PT_sb = [
                small.tile([64, 128], FP, tag=f"PTs{p}", name=f"PT_sb{p}")
                for p in range(NPAIR)
            ]
            FT_sb = [
                small.tile([64, 128], FP, tag=f"FTs{p}", name=f"FT_sb{p}")
                for p in range(NPAIR)
            ]
            for p in range(NPAIR):
                nc.scalar.copy(FT_sb[p][:], FT_ps[p][0:64, :])
                nc.scalar.copy(PT_sb[p][:], PT_ps[p][0:64, :])
            if debug_taps:
                for p in range(NPAIR):
                    nc.sync.dma_start(taps["pt"][64 * p : 64 * p + 64], PT_sb[p][:])
                    nc.sync.dma_start(taps["ft"][64 * p : 64 * p + 64], FT_sb[p][:])


            # SKQ rows: 0:40 sort-q blocks (b0 at 0:8, b1 at 32:40),
            #           64:104 sort-k blocks (b0 at 64:72, b1 at 96:104);
            # one PSUM bank per pair: each holds a long-open accumulation group
            # opened by the constant-term matmul (ready at kernel start) and
            # closed by the PT-part matmul (the only one on the critical tail)
            C104O = NC128
            SQs = []
            RKs = []
            for p in range(NPAIR):
                sk_ps_t = pp.tile([128, 128], FP, tag=f"SKQ{p}", name=f"skq{p}")
                sk_ps = sk_ps_t[0:104, :]
                nc.tensor.matmul(
                    sk_ps,
                    lhsT=ident[0:104, 0:104],
                    rhs=cp_sb[0:104, C104O + 128 * p : C104O + 128 * p + 128],
                    start=True,
                    stop=False,
                    skip_group_check=True,
                )
                nc.tensor.matmul(
                    sk_ps,
                    lhsT=c64_sb[:, 208 + 104 * p : 312 + 104 * p],
                    rhs=FT_sb[p][:],
                    start=False,
                    stop=False,
                    skip_group_check=True,
                )
                nc.tensor.matmul(
                    sk_ps,
                    lhsT=c64_sb[:, 104 * p : 104 * p + 104],
                    rhs=PT_sb[p][:],
                    start=False,
                    stop=True,
                    skip_group_check=True,
                )
                sq_sb = small.tile([40, 128], FP, tag=f"SQ{p}")
                nc.scalar.copy(sq_sb[:], sk_ps[0:40, :])
                rk_sb = small.tile([40, 128], FP, tag=f"RK{p}")
                nc.vector.tensor_copy(rk_sb[:], sk_ps[64:104, :])
                SQs.append(sq_sb)
                RKs.append(rk_sb)

            # R group, 65 logit columns per pair: opened early by an
            # identity-weighted matmul that seeds the bank with the additive
            # causal mask (whose column 0 is the pad-row's constant zero
            # logit); the four sq.sk matmuls then accumulate into columns
            # 1:65 of their quadrants, so the masked logits sit in PSUM with
            # no extra elementwise pass and no separate zero-column handling
            # R groups, one PSUM bank per pair, 72 logit columns: col 0
            # pad, col 1 the pad-row's constant zero logit (both written
            # only by the mask seed), cols 2:66 the sq.sk logits, 66:72
            # pad. The 72 width keeps the partition-64 quadrant writes
            # inside the seed-cleared PSUM zero-region window (64 *
            # width-bytes must be 0 mod 2048) and 8-byte aligned. Separate
            # banks let pair 0's entire softmax run while pair 1 is still
            # streaming; only pair 1's path sits on the tail.
            am72 = cp_sb[:, 384:456]
            mm72 = cp_sb[:, 456:528]
            for p in range(NPAIR):
                R_ps = pp.tile([128, 72], FP, tag=f"R{p}", name=f"R_ps{p}")
                nc.tensor.matmul(
                    R_ps[:],
                    lhsT=ident,
                    rhs=am72,
                    start=True,
                    stop=False,
                    skip_group_check=True,
                )
                nc.tensor.matmul(
                    R_ps[0:64, 2:66],
                    lhsT=SQs[p][0:8, 0:64],
                    rhs=RKs[p][0:8, 0:64],
                    start=False,
                    stop=False,
                    skip_group_check=True,
                )
                nc.tensor.matmul(
                    R_ps[64:128, 2:66],
                    lhsT=SQs[p][32:40, 64:128],
                    rhs=RKs[p][32:40, 64:128],
                    start=False,
                    stop=True,
                    skip_group_check=True,
                )

                # masked softmax over the 65 logits (cols 1:66); pads give
                # exp(NEG) = 0 so they never affect max/sum
                mx = small.tile([128, 1], FP, tag=f"mx{p}", name=f"mx{p}")
                nc.vector.reduce_max(mx[:], R_ps[:], axis=X)
                negm = small.tile([128, 1], FP, tag=f"ngm{p}", name=f"ngm{p}")
                nc.vector.tensor_scalar(
                    negm[:], mx[:], 0.0, -1.0,
                    op0=mybir.AluOpType.max, op1=MULT,
                )
                e_sb = small.tile([128, 72], FP, tag=f"e{p}", name=f"e{p}")
                nc.scalar.activation(
                    e_sb[:], R_ps[:], Exp, bias=negm[:], scale=1.0
                )
                s1 = small.tile([128, 1], FP, tag=f"s1{p}", name=f"s1{p}")
                nc.vector.reduce_sum(s1[:], e_sb[:], axis=X)
                rin = small.tile([128, 1], FP, tag=f"rin{p}", name=f"rin{p}")
                nc.vector.reciprocal(rin[:], s1[:])
                outt = small.tile([128, 72], FP, tag=f"ot{p}", name=f"ot{p}")
                # outt = (e * 1/den) * tril-mask, fused
                nc.vector.scalar_tensor_tensor(
                    outt[:], e_sb[:], rin[:], mm72,
                    op0=MULT,
                    op1=MULT,
                )
                # per-pair output DMA on its own queue so pair 0's issues
                # mid-stream and only pair 1's sits on the tail
                oq = nc.scalar if p == 0 else nc.sync
                oq.dma_start(
                    rout.rearrange("(p b) i c -> (b i) p c", p=2)[:, p, :],
                    outt[:, 1:66],
                )

    nc.compile()
    return nc


def _get_program(t_seq=T, enable_asserts=False):
    key = (t_seq, enable_asserts)
    if key not in _PROG_CACHE:
        _PROG_CACHE[key] = _build_program(t_seq, enable_asserts=enable_asserts)
    return _PROG_CACHE[key]


def _host_constants(core, q_pos_emb, k_pos_emb, Wsq, Wsk, chunk=CHUNK):
    """Single packed per-core constant tensor."""
    f32 = np.float32
    j = np.arange(64, dtype=np.float64)
    s = (1.0 / (chunk * j + 1.0)).astype(f32)  # per-bucket cumavg scale

    tri = np.triu(np.ones((64, 64), f32), k=1)  # [c, j] = 1 iff c < j
    tri_s = tri * s[None, :]
    lmat_s = np.zeros((128, 128), f32)
    lmat_s[0:64, 0:64] = tri_s
    lmat_s[64:128, 64:128] = tri_s
    idents = np.zeros((128, 128), f32)
    idents[np.arange(128), np.arange(128)] = np.concatenate([s, s])
    ident = np.eye(128, dtype=f32)

    rows = np.arange(64)[:, None]
    jj = np.arange(65)[None, :]
    # 72-wide block (same for both pairs): cols 0/66:72 pad (NEG / 0), col
    # 1+j for logit j: valid iff j <= i (j=0 = pad row's constant zero
    # logit), output keeps j < i
    am = np.full((64, 72), NEG, f32)
    am[:, 1:66] = np.where(jj <= rows, 0.0, NEG)
    mm = np.zeros((64, 72), f32)
    mm[:, 1:66] = (jj < rows).astype(f32)
    am72 = np.concatenate([am, am], axis=0)
    mm72 = np.concatenate([mm, mm], axis=0)

    c128 = np.concatenate([lmat_s, idents, ident, am72, mm72], axis=1)

    wq_pt = np.zeros((2, 64, 104), f32)   # [pair][d][sq 0:40 | sk 64:104]
    wq_ft = np.zeros((2, 64, 104), f32)
    cblk = np.zeros((2, 104, 128), f32)   # [pair][skq-row][(b, j)]
    for p in range(NPAIR):
        for b in range(2):
            bh = core * BHC + 2 * p + b
            h = bh % HEADS
            r0 = 32 * b
            wq_pt[p, :, r0 : r0 + 8] = Wsq[0, h, 0:64, :]
            wq_pt[p, :, 64 + r0 : 64 + r0 + 8] = Wsk[0, h, 0:64, :]
            wq_ft[p, :, r0 : r0 + 8] = Wsq[0, h, 64:128, :]
            wq_ft[p, :, 64 + r0 : 64 + r0 + 8] = Wsk[0, h, 64:128, :]
            cq = q_pos_emb[0, h] @ Wsq[0, h, 128:192, :]  # (64, 8)
            ck = k_pos_emb[0, h] @ Wsk[0, h, 128:192, :]
            cblk[p, r0 : r0 + 8, 64 * b : 64 * b + 64] = cq.T
            cblk[p, 64 + r0 : 64 + r0 + 8, 64 * b : 64 * b + 64] = ck.T

    c64 = np.concatenate([wq_pt[0], wq_pt[1], wq_ft[0], wq_ft[1]], axis=1)
    c104 = np.concatenate([cblk[0], cblk[1]], axis=1)
    c104 = np.concatenate([c104, np.zeros((24, NC104), f32)], axis=0)
    cpack = np.concatenate([c128, c104], axis=1)
    assert cpack.shape == (128, NCALL), cpack.shape
    assert c64.shape == (64, NC64), c64.shape
    return {"cpack": cpack, "c64": c64}


def _run(k, q_pos_emb, k_pos_emb, Wsq, Wsk, trace=False, t_seq=T):
    nc = _get_program(t_seq)
    in_maps = []
    for core in range(NCORES):
        cm = _host_constants(
            core, q_pos_emb, k_pos_emb, Wsq, Wsk, chunk=t_seq // BUCKETS
        )
        cm["kin"] = np.ascontiguousarray(k[core * BHC : (core + 1) * BHC])
        in_maps.append(cm)
    res = bass_utils.run_bass_kernel_spmd(
        nc,
        in_maps,
        core_ids=list(range(NCORES)),
        trace=trace,
        **(TRACE_KWARGS if trace else {}),
    )
    global LAST_RESULTS
    LAST_RESULTS = res
    out = np.concatenate([r["rout"] for r in res.results], axis=0)
    return out, res


def kernel(**inputs):
    k = np.asarray(inputs["k"], np.float32)
    q_pos_emb = np.asarray(inputs["q_pos_emb"], np.float32)
    k_pos_emb = np.asarray(inputs["k_pos_emb"], np.float32)
    Wsq = np.asarray(inputs["Wsq"], np.float32)
    Wsk = np.asarray(inputs["Wsk"], np.float32)
    out, _ = _run(k, q_pos_emb, k_pos_emb, Wsq, Wsk, trace=TRACE)
    return out


# revision 18
# speedup vs baseline: 1.1221x; 1.0676x over previous
"""Trainium2 Bass kernel for CausalAttentionSortNet bucket-scoring.

Math (see reference): only `k` feeds the output. For each merged batch*head
slice, the cumulative-average of k is sampled at bucket starts (every 128th
row), which reduces to per-chunk sums + a strictly-triangular prefix matmul.
The rest is tiny per-bucket sort projections and a 64x65 masked softmax.

Sharding: data-parallel over the merged (batch*heads)=32 axis across 8 cores,
4 slices per core, processed as 2 pairs of 2 slices; a pair fills the
128-partition dim as partition=(slice_in_pair, chunk), free=(row, dim) so
every partition's k data is contiguous 32KB HBM runs (the single-queue bulk
stream saturates all 16 DMA engines at ~350 GB/s).

`q` (half of all input bytes) is never read by the reference computation, so
it is not even transferred to the device.

DMA-instruction budget: the hardware exposes ~12 DMA completion semaphores;
an instruction >=12 positions later reuses an earlier one's semaphore and
its issue blocks until that user completes, so the constants ship in two
early-completing DMAs and every bulk tile is uniform (a version with a slow
small-packet constant DMA in the reuse chain stalled the bulk queue 12us).
Chunk first-rows are not a separate DMA: they arrive inside each pair's
first bulk tile, whose in-place fold targets the tile's upper half so row 0
survives for the F-term matmuls.

Per-chunk reduction: each pair's rows stream as sub-tiles of
(16x7, 8, 4, 4) rows. Mid-stream, SBUF port contention caps DVE at roughly
1.7ns/elem and GpSimd at ~2.9ns/elem (vs 1.04/2.0 idle) and LARGER chains
degrade further (superlinear contention), so each sub-tile gets an
INDEPENDENT halving-fold chain (contiguous tensor_adds down to one row ->
its own partial-sum slot) and the chains are statically balanced across
both engines: GpSimd takes pair 1's first six chains, DVE everything else
including every chain near the tail. The PE (otherwise idle) folds every
partial into the scaled-prefix via one matmul per sub-tile against the
tril*scale constant, accumulating in that pair's PSUM bank, opened by the
F*diag(s) seed and closed by the last sub-tile's matmul. Small sub-tiles
stream last so the post-stream tail is two short fold chains plus the
epilogue (projections -> 64x65 masked softmax, batched over both pairs).
"""

from contextlib import ExitStack

import numpy as np

import concourse.bacc as bacc
import concourse.mybir as mybir
import concourse.tile as tile
from concourse import bass_utils

# Problem constants (hardcoded per contract; kernel.py must be self-contained).
B, HEADS, BUCKETS, DIM, DIM_SORT, T = 4, 8, 64, 64, 8, 8192
BH = B * HEADS            # 32 merged batch*head slices
NCORES = 8
BHC = BH // NCORES        # 4 slices per core
NPAIR = BHC // 2          # 2 pairs per core
CHUNK = T // BUCKETS      # 128 rows per bucket
NEG = -1.0e30             # softmax mask value (underflows exp to exactly 0)
FP = mybir.dt.float32

# packed-constant column offsets
NC128 = 128 * 5 + 2
NC64 = 4 * 104
NC104 = 2 * 128
NCALL = NC128 + NC104

# pair-1 fold chains for sub-tiles [0, GP_CHAINS) run on GpSimd; all other
# chains (including every chain near the tail) on the faster DVE
GP_CHAINS = 6

TRACE = False  # set by test.py for profiling runs
TRACE_KWARGS = {}  # extra run_bass_kernel_spmd kwargs for profiling runs
LAST_RESULTS = None  # BassKernelResults of the most recent run

_PROG_CACHE = {}


def _cascade_sizes(chunk):
    # uniform mid-size tiles, small ones last: (16x7, 8, 4, 4) for chunk=128
    assert chunk == 128, "sub-tile schedule is tuned for chunk=128"
    sizes = [16] * 7 + [8, 4, 4]
    assert sum(sizes) == chunk, (sizes, chunk)
    return sizes


def _build_program(t_seq=T, enable_asserts=False, debug_taps=False):
    chunk = t_seq // BUCKETS
    sizes = _cascade_sizes(chunk)
    nsub = len(sizes)

    nc = bacc.Bacc(
        "TRN2",
        target_bir_lowering=False,
        debug=False,
        enable_asserts=enable_asserts,
        num_devices=NCORES,
    )

    def din(name, shape):
        return nc.dram_tensor(name, shape, FP, kind="ExternalInput").ap()

    kin = din("kin", (BHC, t_seq, DIM))
    # packed constants, two DMAs:
    # cpack cols 0:642     c128 = [lmat_s | idents | ident | amask_b | mmask_b | mask0_b]
    # cpack cols 642:898   c104 = per pair (104, 128) cq/ck blocks (rows 104:128 zero)
    # c64 (64, 416)        [wqk_pt_p0 | wqk_pt_p1 | wqk_ft_p0 | wqk_ft_p1]
    cpack = din("cpack", (128, NCALL))
    c64 = din("c64", (64, NC64))
    rout = nc.dram_tensor(
        "rout", (BHC, BUCKETS, BUCKETS + 1), FP, kind="ExternalOutput"
    ).ap()

    X = mybir.AxisListType.X
    Exp = mybir.ActivationFunctionType.Exp
    MULT = mybir.AluOpType.mult

    with tile.TileContext(nc) as tc:
        with ExitStack() as ctx:
            singles = ctx.enter_context(tc.tile_pool(name="singles", bufs=1))
            kpools = [
                ctx.enter_context(tc.tile_pool(name=f"kpool{s}", bufs=2))
                for s in range(nsub)
            ]
            parp = ctx.enter_context(tc.tile_pool(name="parp", bufs=nsub))
            small = ctx.enter_context(tc.tile_pool(name="small", bufs=2))
            pp = ctx.enter_context(tc.tile_pool(name="pp", bufs=1, space="PSUM"))

            cp_sb = singles.tile([128, NCALL], FP, tag="cpack")
            nc.scalar.dma_start(cp_sb[:], cpack)
            c64_sb = singles.tile([64, NC64], FP, tag="c64")
            nc.scalar.dma_start(c64_sb[:], c64)

            # ---- bulk k sub-tile DMAs, single queue, pair 1 leading so
            # its GpSimd chains start first (contiguous rows*256B runs per
            # partition)
            ksrcs = [
                kin[2 * p : 2 * p + 2].rearrange("b (c r) d -> (b c) r d", r=chunk)
                for p in range(NPAIR)
            ]
            kts = {}
            r0 = 0
            for s, rs in enumerate(sizes):
                for p in (1, 0):
                    kt = kpools[s].tile(
                        [128, rs, DIM], FP, tag=f"kt{s}", name=f"kt{s}_{p}"
                    )
                    nc.sync.dma_start(kt[:], ksrcs[p][:, r0 : r0 + rs, :])
                    kts[(p, s)] = kt
                r0 += rs

            lmat_s = cp_sb[:, 0:128]
            idents = cp_sb[:, 128:256]
            ident = cp_sb[:, 256:384]
            # amask (cols 384:512) is consumed directly by the R-group matmul
            mmask_b = cp_sb[:, 512:640].rearrange("q (p j) -> q p j", p=2)
            mask0_b = cp_sb[:, 640:642]

            # ---- PSUM groups, one bank per (pair, tensor): FT_p is F
            # transposed; PT_p is opened by the F*diag(s) seed and closed by
            # that pair's last chunk-sum prefix matmul. F = row 0 of the
            # pair's first bulk tile.
            PT_ps = [
                pp.tile([64, 128], FP, tag=f"PT{p}", name=f"PT_ps{p}")
                for p in range(NPAIR)
            ]
            FT_ps = [
                pp.tile([64, 128], FP, tag=f"FT{p}", name=f"FT_ps{p}")
                for p in range(NPAIR)
            ]
            for p in range(NPAIR):
                nc.tensor.matmul(
                    FT_ps[p][:],
                    lhsT=kts[(p, 0)][:, 0, :],
                    rhs=ident,
                    start=True,
                    stop=True,
                )
                nc.tensor.matmul(
                    PT_ps[p][:],
                    lhsT=kts[(p, 0)][:, 0, :],
                    rhs=idents,
                    start=True,
                    stop=False,
                )

            # ---- per-sub-tile fold chains + per-sub-tile prefix matmuls.
            # Each (pair, sub-tile) folds independently down to one row (the
            # first fold targets the upper half so row 0 survives in tile 0),
            # writing its own partial-sum slot; the PE folds every partial
            # into the pair's scaled-prefix PSUM bank as it appears.
            pars = [
                parp.tile([128, NPAIR, DIM], FP, tag=f"par{s}", name=f"par{s}")
                for s in range(nsub)
            ]
            for s, rs in enumerate(sizes):
                for p in (1, 0):
                    t = kts[(p, s)]
                    e = nc.gpsimd if (p == 1 and s < GP_CHAINS) else nc.vector
                    h = rs // 2
                    e.tensor_add(t[:, h:rs, :], t[:, h:rs, :], t[:, 0:h, :])
                    lo, xr = h, h
                    while xr > 2:
                        hh = xr // 2
                        e.tensor_add(
                            t[:, lo : lo + hh, :],
                            t[:, lo : lo + hh, :],
                            t[:, lo + hh : lo + xr, :],
                        )
                        xr = hh
                    e.tensor_add(
                        pars[s][:, p, :], t[:, lo, :], t[:, lo + 1, :]
                    )
                    nc.tensor.matmul(
                        PT_ps[p][:],
                        lhsT=pars[s][:, p, :],
                        rhs=lmat_s,
                        start=False,
                        stop=s == nsub - 1,
                    )

            # ---- sort projections (per pair), batched softmax (both pairs)
            PT_sb = [
                small.tile([64, 128], FP, tag=f"PTs{p}", name=f"PT_sb{p}")
                for p in range(NPAIR)
            ]
            FT_sb = [
                small.tile([64, 128], FP, tag=f"FTs{p}", name=f"FT_sb{p}")
                for p in range(NPAIR)
            ]
            for p in range(NPAIR):
                nc.scalar.copy(FT_sb[p][:], FT_ps[p][:])
                nc.scalar.copy(PT_sb[p][:], PT_ps[p][:])

            # SKQ rows: 0:40 sort-q blocks (b0 at 0:8, b1 at 32:40),
            #           64:104 sort-k blocks (b0 at 64:72, b1 at 96:104);
            # one PSUM bank per pair: each holds a long-open accumulation group
            # opened by the constant-term matmul (ready at kernel start) and
            # closed by the PT-part matmul (the only one on the critical tail)
            C104O = NC128
            SQs = []
            RKs = []
            for p in range(NPAIR):
                sk_ps_t = pp.tile([104, 128], FP, tag=f"SKQ{p}")
                sk_ps = sk_ps_t[:]
                nc.tensor.matmul(
                    sk_ps,
                    lhsT=ident[0:104, 0:104],
                    rhs=cp_sb[0:104, C104O + 128 * p : C104O + 128 * p + 128],
                    start=True,
                    stop=False,
                    skip_group_check=True,
                )
                nc.tensor.matmul(
                    sk_ps,
                    lhsT=c64_sb[:, 208 + 104 * p : 312 + 104 * p],
                    rhs=FT_sb[p][:],
                    start=False,
                    stop=False,
                    skip_group_check=True,
                )
                nc.tensor.matmul(
                    sk_ps,
                    lhsT=c64_sb[:, 104 * p : 104 * p + 104],
                    rhs=PT_sb[p][:],
                    start=False,
                    stop=True,
                    skip_group_check=True,
                )
                sq_sb = small.tile([40, 128], FP, tag=f"SQ{p}")
                nc.scalar.copy(sq_sb[:], sk_ps[0:40, :])
                rk_sb = small.tile([40, 128], FP, tag=f"RK{p}")
                nc.vector.tensor_copy(rk_sb[:], sk_ps[64:104, :])
                SQs.append(sq_sb)
                RKs.append(rk_sb)

            # R group: opened early by an identity-weighted matmul that seeds
            # the bank with the additive causal mask; the four sq.sk matmuls
            # then accumulate into their quadrants, so the masked logits sit
            # in PSUM with no extra elementwise pass
            R_ps = pp.tile([128, 128], FP, tag="R")
            nc.tensor.matmul(
                R_ps[:],
                lhsT=ident,
                rhs=cp_sb[:, 384:512],
                start=True,
                stop=False,
                skip_group_check=True,
            )
            for p in range(NPAIR):
                nc.tensor.matmul(
                    R_ps[0:64, 64 * p : 64 * p + 64],
                    lhsT=SQs[p][0:8, 0:64],
                    rhs=RKs[p][0:8, 0:64],
                    start=False,
                    stop=False,
                    skip_group_check=True,
                )
                nc.tensor.matmul(
                    R_ps[64:128, 64 * p : 64 * p + 64],
                    lhsT=SQs[p][32:40, 64:128],
                    rhs=RKs[p][32:40, 64:128],
                    start=False,
                    stop=p == NPAIR - 1,
                    skip_group_check=True,
                )

            # masked softmax over 65 logits (implicit zero-logit column 0),
            # both pairs batched along the free axis: cols = (pair, j)
            Rm = R_ps[:].rearrange("q (p j) -> q p j", p=2)
            mx = small.tile([128, 2], FP, tag="mx")
            nc.vector.reduce_max(mx[:], Rm, axis=X)
            negm = small.tile([128, 2], FP, tag="negm")
            nc.vector.tensor_scalar(
                negm[:], mx[:], 0.0, -1.0,
                op0=mybir.AluOpType.max, op1=MULT,
            )
            e0 = small.tile([128, 2], FP, tag="e0")
            nc.scalar.activation(e0[:], negm[:], Exp)
            e_sb = small.tile([128, 2, 64], FP, tag="e")
            for p in range(NPAIR):
                nc.scalar.activation(
                    e_sb[:, p, :], R_ps[:, 64 * p : 64 * p + 64], Exp,
                    bias=negm[:, p : p + 1], scale=1.0,
                )
            s1 = small.tile([128, 2], FP, tag="s1")
            nc.vector.reduce_sum(s1[:], e_sb[:], axis=X)
            den = small.tile([128, 2], FP, tag="den")
            nc.vector.tensor_add(den[:], s1[:], e0[:])
            rin = small.tile([128, 2], FP, tag="rin")
            nc.vector.reciprocal(rin[:], den[:])
            outt = small.tile([128, 2, BUCKETS + 1], FP, tag="outt")
            for p in range(NPAIR):
                # outt = (e * 1/den) * tril-mask, fused
                nc.vector.scalar_tensor_tensor(
                    outt[:, p, 1:],
                    e_sb[:, p, :],
                    rin[:, p : p + 1],
                    mmask_b[:, p, :],
                    op0=MULT,
                    op1=MULT,
                )
            t0 = small.tile([128, 2], FP, tag="t0")
            nc.vector.tensor_mul(t0[:], e0[:], mask0_b)
            nc.vector.tensor_mul(outt[:, :, 0], t0[:], rin[:])
            nc.sync.dma_start(
                rout.rearrange("(p b) i c -> (b i) p c", p=2), outt[:]
            )

    nc.compile()
    return nc


def _get_program(t_seq=T, enable_asserts=False):
    key = (t_seq, enable_asserts)
    if key not in _PROG_CACHE:
        _PROG_CACHE[key] = _build_program(t_seq, enable_asserts=enable_asserts)
    return _PROG_CACHE[key]


def _host_constants(core, q_pos_emb, k_pos_emb, Wsq, Wsk, chunk=CHUNK):
    """Packed per-core constant tensors (two DMAs)."""
    f32 = np.float32
    j = np.arange(64, dtype=np.float64)
    s = (1.0 / (chunk * j + 1.0)).astype(f32)  # per-bucket cumavg scale

    tri = np.triu(np.ones((64, 64), f32), k=1)  # [c, j] = 1 iff c < j
    tri_s = tri * s[None, :]
    lmat_s = np.zeros((128, 128), f32)
    lmat_s[0:64, 0:64] = tri_s
    lmat_s[64:128, 64:128] = tri_s
    idents = np.zeros((128, 128), f32)
    idents[np.arange(128), np.arange(128)] = np.concatenate([s, s])
    ident = np.eye(128, dtype=f32)

    rows = np.arange(64)[:, None]
    cols = np.arange(64)[None, :]
    am = np.where(cols < rows, 0.0, NEG).astype(f32)       # softmax additive mask
    mm = (cols <= rows - 2).astype(f32)                    # output tril(-1) mask
    amask_b = np.concatenate([am, am], axis=1)
    amask_b = np.concatenate([amask_b, amask_b], axis=0)
    mmask_b = np.concatenate([mm, mm], axis=1)
    mmask_b = np.concatenate([mmask_b, mmask_b], axis=0)
    m0 = (np.arange(64) > 0).astype(f32).reshape(64, 1)
    mask0_b = np.concatenate([np.concatenate([m0, m0], 1)] * 2, 0)

    c128 = np.concatenate([lmat_s, idents, ident, amask_b, mmask_b, mask0_b], axis=1)

    wq_pt = np.zeros((2, 64, 104), f32)   # [pair][d][sq 0:40 | sk 64:104]
    wq_ft = np.zeros((2, 64, 104), f32)
    cblk = np.zeros((2, 104, 128), f32)   # [pair][skq-row][(b, j)]
    for p in range(NPAIR):
        for b in range(2):
            bh = core * BHC + 2 * p + b
            h = bh % HEADS
            r0 = 32 * b
            wq_pt[p, :, r0 : r0 + 8] = Wsq[0, h, 0:64, :]
            wq_pt[p, :, 64 + r0 : 64 + r0 + 8] = Wsk[0, h, 0:64, :]
            wq_ft[p, :, r0 : r0 + 8] = Wsq[0, h, 64:128, :]
            wq_ft[p, :, 64 + r0 : 64 + r0 + 8] = Wsk[0, h, 64:128, :]
            cq = q_pos_emb[0, h] @ Wsq[0, h, 128:192, :]  # (64, 8)
            ck = k_pos_emb[0, h] @ Wsk[0, h, 128:192, :]
            cblk[p, r0 : r0 + 8, 64 * b : 64 * b + 64] = cq.T
            cblk[p, 64 + r0 : 64 + r0 + 8, 64 * b : 64 * b + 64] = ck.T

    c64 = np.concatenate([wq_pt[0], wq_pt[1], wq_ft[0], wq_ft[1]], axis=1)
    c104 = np.concatenate([cblk[0], cblk[1]], axis=1)
    c104 = np.concatenate([c104, np.zeros((24, NC104), f32)], axis=0)
    cpack = np.concatenate([c128, c104], axis=1)
    assert cpack.shape == (128, NCALL), cpack.shape
    assert c64.shape == (64, NC64), c64.shape
    return {"cpack": cpack, "c64": c64}


def _run(k, q_pos_emb, k_pos_emb, Wsq, Wsk, trace=False, t_seq=T):
    nc = _get_program(t_seq)
    in_maps = []
    for core in range(NCORES):
        cm = _host_constants(
            core, q_pos_emb, k_pos_emb, Wsq, Wsk, chunk=t_seq // BUCKETS
        )
        cm["kin"] = np.ascontiguousarray(k[core * BHC : (core + 1) * BHC])
        in_maps.append(cm)
    res = bass_utils.run_bass_kernel_spmd(
        nc,
        in_maps,
        core_ids=list(range(NCORES)),
        trace=trace,
        **(TRACE_KWARGS if trace else {}),
    )
    global LAST_RESULTS
    LAST_RESULTS = res
    out = np.concatenate([r["rout"] for r in res.results], axis=0)
    return out, res


def kernel(**inputs):
    k = np.asarray(inputs["k"], np.float32)
    q_pos_emb = np.asarray(inputs["q_pos_emb"], np.float32)
    k_pos_emb = np.asarray(inputs["k_pos_emb"], np.float32)
    Wsq = np.asarray(inputs["Wsq"], np.float32)
    Wsk = np.asarray(inputs["Wsk"], np.float32)
    out, _ = _run(k, q_pos_emb, k_pos_emb, Wsq, Wsk, trace=TRACE)
    return out


# revision 21
# speedup vs baseline: 1.1285x; 1.0057x over previous
"""Trainium2 Bass kernel for CausalAttentionSortNet bucket-scoring.

Math (see reference): only `k` feeds the output. For each merged batch*head
slice, the cumulative-average of k is sampled at bucket starts (every 128th
row), which reduces to per-chunk sums + a strictly-triangular prefix matmul.
The rest is tiny per-bucket sort projections and a 64x65 masked softmax.

Sharding: data-parallel over the merged (batch*heads)=32 axis across 8 cores,
4 slices per core, processed as 2 pairs of 2 slices; a pair fills the
128-partition dim as partition=(slice_in_pair, chunk), free=(row, dim) so
every partition's k data is contiguous 32KB HBM runs (the single-queue bulk
stream saturates all 16 DMA engines at ~350 GB/s).

`q` (half of all input bytes) is never read by the reference computation, so
it is not even transferred to the device.

DMA-instruction budget: the hardware exposes ~12 DMA completion semaphores;
an instruction >=12 positions later reuses an earlier one's semaphore and
its issue blocks until that user completes, so the constants ship in two
early-completing DMAs and every bulk tile is uniform (a version with a slow
small-packet constant DMA in the reuse chain stalled the bulk queue 12us).
Chunk first-rows are not a separate DMA: they arrive inside each pair's
first bulk tile, whose in-place fold targets the tile's upper half so row 0
survives for the F-term matmuls.

Per-chunk reduction: each pair's rows stream as sub-tiles of
(16x7, 8, 4, 4) rows. Mid-stream, SBUF port contention caps DVE at roughly
1.7ns/elem and GpSimd at ~2.9ns/elem (vs 1.04/2.0 idle) and LARGER chains
degrade further (superlinear contention), so each sub-tile gets an
INDEPENDENT halving-fold chain (contiguous tensor_adds down to one row ->
its own partial-sum slot) and the chains are statically balanced across
both engines: GpSimd takes pair 1's first six chains, DVE everything else
including every chain near the tail. The PE (otherwise idle) folds every
partial into the scaled-prefix via one matmul per sub-tile against the
tril*scale constant, accumulating in that pair's PSUM bank, opened by the
F*diag(s) seed and closed by the last sub-tile's matmul. Small sub-tiles
stream last so the post-stream tail is two short fold chains plus the
epilogue (projections -> 64x65 masked softmax, batched over both pairs).
"""

from contextlib import ExitStack

import numpy as np

import concourse.bacc as bacc
import concourse.mybir as mybir
import concourse.tile as tile
from concourse import bass_utils

# Problem constants (hardcoded per contract; kernel.py must be self-contained).
B, HEADS, BUCKETS, DIM, DIM_SORT, T = 4, 8, 64, 64, 8, 8192
BH = B * HEADS            # 32 merged batch*head slices
NCORES = 8
BHC = BH // NCORES        # 4 slices per core
NPAIR = BHC // 2          # 2 pairs per core
CHUNK = T // BUCKETS      # 128 rows per bucket
NEG = -1.0e30             # softmax mask value (underflows exp to exactly 0)
FP = mybir.dt.float32

# packed-constant column offsets: cpack = [lmat_s | s2col], caux packs the
# nonzero rows of the cq/ck seed blocks plus their row-scatter matrix; the
# other structural constants (identity, diag(s), masks) are built on-chip
# by GpSimd affine_selects during its pre-stream idle window.
NC128 = 128 + 1
NC64 = 4 * 104
NCAUX = 2 * 128 + 104
NCALL = NC128

# pair-1 fold chains for sub-tiles [0, GP_CHAINS) run on GpSimd; all other
# chains (including every chain near the tail) on the faster DVE
GP_CHAINS = 6

TRACE = False  # set by test.py for profiling runs
TRACE_KWARGS = {}  # extra run_bass_kernel_spmd kwargs for profiling runs
LAST_RESULTS = None  # BassKernelResults of the most recent run

_PROG_CACHE = {}


def _cascade_sizes(chunk):
    # uniform mid-size tiles, small ones last: (16x7, 8, 4, 4) for chunk=128
    assert chunk == 128, "sub-tile schedule is tuned for chunk=128"
    sizes = [16] * 7 + [8, 4, 4]
    assert sum(sizes) == chunk, (sizes, chunk)
    return sizes


def _build_program(t_seq=T, enable_asserts=False, debug_taps=False):
    chunk = t_seq // BUCKETS
    sizes = _cascade_sizes(chunk)
    nsub = len(sizes)

    nc = bacc.Bacc(
        "TRN2",
        target_bir_lowering=False,
        debug=False,
        enable_asserts=enable_asserts,
        num_devices=NCORES,
    )

    def din(name, shape):
        return nc.dram_tensor(name, shape, FP, kind="ExternalInput").ap()

    kin = din("kin", (BHC, t_seq, DIM))
    # packed constants, three small DMAs:
    # cpack (128, 129)  [lmat_s | s-column]
    # c64   (64, 416)   [wqk_pt_p0 | wqk_pt_p1 | wqk_ft_p0 | wqk_ft_p1]
    # caux  (32, 360)   [c104 packed nonzero rows (256) | row-scatter P (104)]
    cpack = din("cpack", (128, NCALL))
    c64 = din("c64", (64, NC64))
    caux = din("caux", (32, NCAUX))
    rout = nc.dram_tensor(
        "rout", (BHC, BUCKETS, BUCKETS + 1), FP, kind="ExternalOutput"
    ).ap()

    X = mybir.AxisListType.X
    Exp = mybir.ActivationFunctionType.Exp
    MULT = mybir.AluOpType.mult

    with tile.TileContext(nc) as tc:
        with ExitStack() as ctx:
            singles = ctx.enter_context(tc.tile_pool(name="singles", bufs=1))
            kpools = [
                ctx.enter_context(tc.tile_pool(name=f"kpool{s}", bufs=2))
                for s in range(nsub)
            ]
            parp = ctx.enter_context(tc.tile_pool(name="parp", bufs=nsub))
            small = ctx.enter_context(tc.tile_pool(name="small", bufs=2))
            pp = ctx.enter_context(tc.tile_pool(name="pp", bufs=1, space="PSUM"))

            cp_sb = singles.tile([128, NCALL], FP, tag="cpack")
            nc.scalar.dma_start(cp_sb[:], cpack)
            c64_sb = singles.tile([64, NC64], FP, tag="c64")
            nc.scalar.dma_start(c64_sb[:], c64)
            caux_sb = singles.tile([32, NCAUX], FP, tag="caux")
            nc.scalar.dma_start(caux_sb[:], caux)

            # ---- bulk k sub-tile DMAs, single queue, pair 1 leading so
            # its GpSimd chains start first (contiguous rows*256B runs per
            # partition)
            ksrcs = [
                kin[2 * p : 2 * p + 2].rearrange("b (c r) d -> (b c) r d", r=chunk)
                for p in range(NPAIR)
            ]
            kts = {}
            r0 = 0
            for s, rs in enumerate(sizes):
                for p in (1, 0):
                    kt = kpools[s].tile(
                        [128, rs, DIM], FP, tag=f"kt{s}", name=f"kt{s}_{p}"
                    )
                    nc.sync.dma_start(kt[:], ksrcs[p][:, r0 : r0 + rs, :])
                    kts[(p, s)] = kt
                r0 += rs

            lmat_s = cp_sb[:, 0:128]
            s2col = cp_sb[:, 128:129]

            # on-chip structural constants, built by GpSimd (idle until the
            # first bulk tile lands ~11us in) + one Scalar scale op:
            #   eye      128x128 identity
            #   idents   diag(s) (cumavg scales)
            #   am68/mm68: additive causal mask / tril(-1) output mask in the
            #   68-wide-per-pair logit layout (col 0 pad, col 1+j logit j
            #   valid iff j <= row, cols 66:68 pad)
            # neuronxcc implements only is_ge / is_gt / not_equal for
            # affine_select, so every predicate is phrased with those
            GE, GT, NE = (
                mybir.AluOpType.is_ge,
                mybir.AluOpType.is_gt,
                mybir.AluOpType.not_equal,
            )
            eye_sb = singles.tile([128, 128], FP, tag="eye")
            nc.gpsimd.memset(eye_sb[:], 0.0)
            # where(c - i != 0, 0, fill=1) = identity
            nc.gpsimd.affine_select(
                eye_sb[:], eye_sb[:], [[1, 128]], NE, 1.0,
                base=0, channel_multiplier=-1,
            )
            idents_sb = singles.tile([128, 128], FP, tag="idents")
            nc.scalar.activation(
                idents_sb[:], eye_sb[:], mybir.ActivationFunctionType.Copy,
                scale=s2col,
            )
            am_sb = singles.tile([128, 2, 68], FP, tag="am68")
            nc.gpsimd.memset(am_sb[:], 0.0)
            for b in range(2):
                # keep 0 where row - pos + 1 >= 0 (valid logit), else NEG
                nc.gpsimd.affine_select(
                    am_sb[64 * b : 64 * b + 64], am_sb[64 * b : 64 * b + 64],
                    [[0, 2], [-1, 68]], GE, NEG,
                    base=1, channel_multiplier=1,
                )
            # pad columns: keep pos - 1 >= 0, keep 65 - pos >= 0, else NEG
            nc.gpsimd.affine_select(
                am_sb[:], am_sb[:], [[0, 2], [1, 68]], GE, NEG, base=-1,
                channel_multiplier=0,
            )
            nc.gpsimd.affine_select(
                am_sb[:], am_sb[:], [[0, 2], [-1, 68]], GE, NEG, base=65,
                channel_multiplier=0,
            )
            mm_sb = singles.tile([128, 2, 68], FP, tag="mm68")
            nc.gpsimd.memset(mm_sb[:], 1.0)
            for b in range(2):
                # keep 1 where row - pos + 1 > 0 (output tril(-1)), else 0
                nc.gpsimd.affine_select(
                    mm_sb[64 * b : 64 * b + 64], mm_sb[64 * b : 64 * b + 64],
                    [[0, 2], [-1, 68]], GT, 0.0,
                    base=1, channel_multiplier=1,
                )
            nc.gpsimd.affine_select(
                mm_sb[:], mm_sb[:], [[0, 2], [1, 68]], GE, 0.0, base=-1,
                channel_multiplier=0,
            )
            nc.gpsimd.affine_select(
                mm_sb[:], mm_sb[:], [[0, 2], [-1, 68]], GE, 0.0, base=65,
                channel_multiplier=0,
            )
            idents = idents_sb[:]
            ident = eye_sb[:]
            mmask_b = mm_sb[:]

            # ---- PSUM groups, one bank per (pair, tensor): FT_p is F
            # transposed; PT_p is opened by the F*diag(s) seed and closed by
            # that pair's last chunk-sum prefix matmul. F = row 0 of the
            # pair's first bulk tile.
            PT_ps = [
                pp.tile([64, 128], FP, tag=f"PT{p}", name=f"PT_ps{p}")
                for p in range(NPAIR)
            ]
            FT_ps = [
                pp.tile([64, 128], FP, tag=f"FT{p}", name=f"FT_ps{p}")
                for p in range(NPAIR)
            ]
            for p in range(NPAIR):
                nc.tensor.matmul(
                    FT_ps[p][:],
                    lhsT=kts[(p, 0)][:, 0, :],
                    rhs=ident,
                    start=True,
                    stop=True,
                )
                nc.tensor.matmul(
                    PT_ps[p][:],
                    lhsT=kts[(p, 0)][:, 0, :],
                    rhs=idents,
                    start=True,
                    stop=False,
                )

            # ---- per-sub-tile fold chains + per-sub-tile prefix matmuls.
            # Each (pair, sub-tile) folds independently down to one row (the
            # first fold targets the upper half so row 0 survives in tile 0),
            # writing its own partial-sum slot; the PE folds every partial
            # into the pair's scaled-prefix PSUM bank as it appears.
            pars = [
                parp.tile([128, NPAIR, DIM], FP, tag=f"par{s}", name=f"par{s}")
                for s in range(nsub)
            ]
            for s, rs in enumerate(sizes):
                for p in (1, 0):
                    t = kts[(p, s)]
                    e = nc.gpsimd if (p == 1 and s < GP_CHAINS) else nc.vector
                    h = rs // 2
                    e.tensor_add(t[:, h:rs, :], t[:, h:rs, :], t[:, 0:h, :])
                    lo, xr = h, h
                    while xr > 2:
                        hh = xr // 2
                        e.tensor_add(
                            t[:, lo : lo + hh, :],
                            t[:, lo : lo + hh, :],
                            t[:, lo + hh : lo + xr, :],
                        )
                        xr = hh
                    e.tensor_add(
                        pars[s][:, p, :], t[:, lo, :], t[:, lo + 1, :]
                    )
                    nc.tensor.matmul(
                        PT_ps[p][:],
                        lhsT=pars[s][:, p, :],
                        rhs=lmat_s,
                        start=False,
                        stop=s == nsub - 1,
                    )

            # ---- sort projections (per pair), batched softmax (both pairs)
            PT_sb = [
                small.tile([64, 128], FP, tag=f"PTs{p}", name=f"PT_sb{p}")
                for p in range(NPAIR)
            ]
            FT_sb = [
                small.tile([64, 128], FP, tag=f"FTs{p}", name=f"FT_sb{p}")
                for p in range(NPAIR)
            ]
            for p in range(NPAIR):
                nc.scalar.copy(FT_sb[p][:], FT_ps[p][:])
                nc.scalar.copy(PT_sb[p][:], PT_ps[p][:])

            # SKQ rows: 0:40 sort-q blocks (b0 at 0:8, b1 at 32:40),
            #           64:104 sort-k blocks (b0 at 64:72, b1 at 96:104);
            # one PSUM bank per pair: each holds a long-open accumulation group
            # opened by the constant-term matmul (ready at kernel start) and
            # closed by the PT-part matmul (the only one on the critical tail)
            SQs = []
            RKs = []
            for p in range(NPAIR):
                sk_ps_t = pp.tile([104, 128], FP, tag=f"SKQ{p}")
                sk_ps = sk_ps_t[:]
                nc.tensor.matmul(
                    sk_ps,
                    lhsT=caux_sb[:, 256:360],
                    rhs=caux_sb[:, 128 * p : 128 * p + 128],
                    start=True,
                    stop=False,
                    skip_group_check=True,
                )
                nc.tensor.matmul(
                    sk_ps,
                    lhsT=c64_sb[:, 208 + 104 * p : 312 + 104 * p],
                    rhs=FT_sb[p][:],
                    start=False,
                    stop=False,
                    skip_group_check=True,
                )
                nc.tensor.matmul(
                    sk_ps,
                    lhsT=c64_sb[:, 104 * p : 104 * p + 104],
                    rhs=PT_sb[p][:],
                    start=False,
                    stop=True,
                    skip_group_check=True,
                )
                sq_sb = small.tile([40, 128], FP, tag=f"SQ{p}")
                nc.scalar.copy(sq_sb[:], sk_ps[0:40, :])
                rk_sb = small.tile([40, 128], FP, tag=f"RK{p}")
                nc.vector.tensor_copy(rk_sb[:], sk_ps[64:104, :])
                SQs.append(sq_sb)
                RKs.append(rk_sb)

            # R group: opened early by an identity-weighted matmul that seeds
            # the bank with the additive causal mask; the four sq.sk matmuls
            # then accumulate into their quadrants, so the masked logits sit
            # in PSUM with no extra elementwise pass
            # 68-wide per-pair blocks: col 0 pad, col 1 the pad-row's
            # constant zero logit (both from the mask seed), cols 2:66 the
            # sq.sk logits, 66:68 pad; width 68 keeps the partition-64
            # quadrant writes aligned to the PSUM zero-region window
            R_ps = pp.tile([128, 2 * 68], FP, tag="R")
            nc.tensor.matmul(
                R_ps[:],
                lhsT=ident,
                rhs=am_sb[:].rearrange("q p j -> q (p j)"),
                start=True,
                stop=False,
                skip_group_check=True,
            )
            for p in range(NPAIR):
                nc.tensor.matmul(
                    R_ps[0:64, 68 * p + 2 : 68 * p + 66],
                    lhsT=SQs[p][0:8, 0:64],
                    rhs=RKs[p][0:8, 0:64],
                    start=False,
                    stop=False,
                    skip_group_check=True,
                )
                nc.tensor.matmul(
                    R_ps[64:128, 68 * p + 2 : 68 * p + 66],
                    lhsT=SQs[p][32:40, 64:128],
                    rhs=RKs[p][32:40, 64:128],
                    start=False,
                    stop=p == NPAIR - 1,
                    skip_group_check=True,
                )

            # masked softmax over the 65 logits (cols 1:66 of each block;
            # pad cols give exp(NEG) = 0), both pairs batched: cols = (p, j)
            Rm = R_ps[:].rearrange("q (p j) -> q p j", p=2)
            mx = small.tile([128, 2], FP, tag="mx")
            nc.vector.reduce_max(mx[:], Rm, axis=X)
            negm = small.tile([128, 2], FP, tag="negm")
            nc.vector.tensor_scalar(
                negm[:], mx[:], 0.0, -1.0,
                op0=mybir.AluOpType.max, op1=MULT,
            )
            e_sb = small.tile([128, 2, 68], FP, tag="e")
            for p in range(NPAIR):
                nc.scalar.activation(
                    e_sb[:, p, :], R_ps[:, 68 * p : 68 * p + 68], Exp,
                    bias=negm[:, p : p + 1], scale=1.0,
                )
            s1 = small.tile([128, 2], FP, tag="s1")
            nc.vector.reduce_sum(s1[:], e_sb[:], axis=X)
            rin = small.tile([128, 2], FP, tag="rin")
            nc.vector.reciprocal(rin[:], s1[:])
            outt = small.tile([128, 2, 68], FP, tag="outt")
            for p in range(NPAIR):
                # outt = (e * 1/den) * tril-mask, fused
                nc.vector.scalar_tensor_tensor(
                    outt[:, p, :],
                    e_sb[:, p, :],
                    rin[:, p : p + 1],
                    mmask_b[:, p, :],
                    op0=MULT,
                    op1=MULT,
                )
            nc.sync.dma_start(
                rout.rearrange("(p b) i c -> (b i) p c", p=2),
                outt[:, :, 1:66],
            )

    nc.compile()
    return nc


def _get_program(t_seq=T, enable_asserts=False):
    key = (t_seq, enable_asserts)
    if key not in _PROG_CACHE:
        _PROG_CACHE[key] = _build_program(t_seq, enable_asserts=enable_asserts)
    return _PROG_CACHE[key]


def _host_constants(core, q_pos_emb, k_pos_emb, Wsq, Wsk, chunk=CHUNK):
    """Packed per-core constant tensors (two DMAs)."""
    f32 = np.float32
    j = np.arange(64, dtype=np.float64)
    s = (1.0 / (chunk * j + 1.0)).astype(f32)  # per-bucket cumavg scale

    tri = np.triu(np.ones((64, 64), f32), k=1)  # [c, j] = 1 iff c < j
    tri_s = tri * s[None, :]
    lmat_s = np.zeros((128, 128), f32)
    lmat_s[0:64, 0:64] = tri_s
    lmat_s[64:128, 64:128] = tri_s
    idents = np.zeros((128, 128), f32)
    idents[np.arange(128), np.arange(128)] = np.concatenate([s, s])
    ident = np.eye(128, dtype=f32)

    c128 = np.concatenate(
        [lmat_s, np.concatenate([s, s]).reshape(128, 1)], axis=1
    )

    wq_pt = np.zeros((2, 64, 104), f32)   # [pair][d][sq 0:40 | sk 64:104]
    wq_ft = np.zeros((2, 64, 104), f32)
    # c104 packed: only the 4 nonzero 8-row bands (SKQ rows 0:8, 32:40,
    # 64:72, 96:104) as 32 rows, plus the (32 -> 104) row-scatter matrix P
    c104p = np.zeros((2, 32, 128), f32)
    P = np.zeros((32, 104), f32)
    for r in range(32):
        P[r, 32 * (r // 8) + (r % 8)] = 1.0
    for p in range(NPAIR):
        for b in range(2):
            bh = core * BHC + 2 * p + b
            h = bh % HEADS
            r0 = 32 * b
            wq_pt[p, :, r0 : r0 + 8] = Wsq[0, h, 0:64, :]
            wq_pt[p, :, 64 + r0 : 64 + r0 + 8] = Wsk[0, h, 0:64, :]
            wq_ft[p, :, r0 : r0 + 8] = Wsq[0, h, 64:128, :]
            wq_ft[p, :, 64 + r0 : 64 + r0 + 8] = Wsk[0, h, 64:128, :]
            cq = q_pos_emb[0, h] @ Wsq[0, h, 128:192, :]  # (64, 8)
            ck = k_pos_emb[0, h] @ Wsk[0, h, 128:192, :]
            c104p[p, 8 * b : 8 * b + 8, 64 * b : 64 * b + 64] = cq.T
            c104p[p, 16 + 8 * b : 24 + 8 * b, 64 * b : 64 * b + 64] = ck.T

    c64 = np.concatenate([wq_pt[0], wq_pt[1], wq_ft[0], wq_ft[1]], axis=1)
    caux = np.concatenate([c104p[0], c104p[1], P], axis=1)
    assert c128.shape == (128, NCALL), c128.shape
    assert c64.shape == (64, NC64), c64.shape
    assert caux.shape == (32, NCAUX), caux.shape
    return {"cpack": c128, "c64": c64, "caux": caux}


def _run(k, q_pos_emb, k_pos_emb, Wsq, Wsk, trace=False, t_seq=T):
    nc = _get_program(t_seq)
    in_maps = []
    for core in range(NCORES):
        cm = _host_constants(
            core, q_pos_emb, k_pos_emb, Wsq, Wsk, chunk=t_seq // BUCKETS
        )
        cm["kin"] = np.ascontiguousarray(k[core * BHC : (core + 1) * BHC])
        in_maps.append(cm)
    res = bass_utils.run_bass_kernel_spmd(
        nc,
        in_maps,
        core_ids=list(range(NCORES)),
        trace=trace,
        **(TRACE_KWARGS if trace else {}),
    )
    global LAST_RESULTS
    LAST_RESULTS = res
    out = np.concatenate([r["rout"] for r in res.results], axis=0)
    return out, res


def kernel(**inputs):
    k = np.asarray(inputs["k"], np.float32)
    q_pos_emb = np.asarray(inputs["q_pos_emb"], np.float32)
    k_pos_emb = np.asarray(inputs["k_pos_emb"], np.float32)
    Wsq = np.asarray(inputs["Wsq"], np.float32)
    Wsk = np.asarray(inputs["Wsk"], np.float32)
    out, _ = _run(k, q_pos_emb, k_pos_emb, Wsq, Wsk, trace=TRACE)
    return out


# revision 24
# speedup vs baseline: 1.1351x; 1.0058x over previous
"""Trainium2 Bass kernel for CausalAttentionSortNet bucket-scoring.

Math (see reference): only `k` feeds the output. For each merged batch*head
slice, the cumulative-average of k is sampled at bucket starts (every 128th
row), which reduces to per-chunk sums + a strictly-triangular prefix matmul.
The rest is tiny per-bucket sort projections and a 64x65 masked softmax.

Sharding: data-parallel over the merged (batch*heads)=32 axis across 8 cores,
4 slices per core, processed as 2 pairs of 2 slices; a pair fills the
128-partition dim as partition=(slice_in_pair, chunk), free=(row, dim) so
every partition's k data is contiguous 32KB HBM runs (the single-queue bulk
stream saturates all 16 DMA engines at ~350 GB/s).

`q` (half of all input bytes) is never read by the reference computation, so
it is not even transferred to the device.

DMA-instruction budget: the hardware exposes ~12 DMA completion semaphores;
an instruction >=12 positions later reuses an earlier one's semaphore and
its issue blocks until that user completes, so the constants ship in two
early-completing DMAs and every bulk tile is uniform (a version with a slow
small-packet constant DMA in the reuse chain stalled the bulk queue 12us).
Chunk first-rows are not a separate DMA: they arrive inside each pair's
first bulk tile, whose in-place fold targets the tile's upper half so row 0
survives for the F-term matmuls.

Per-chunk reduction: each pair's rows stream as sub-tiles of
(16x7, 8, 4, 4) rows. Mid-stream, SBUF port contention caps DVE at roughly
1.7ns/elem and GpSimd at ~2.9ns/elem (vs 1.04/2.0 idle) and LARGER chains
degrade further (superlinear contention), so each sub-tile gets an
INDEPENDENT halving-fold chain (contiguous tensor_adds down to one row ->
its own partial-sum slot) and the chains are statically balanced across
both engines: GpSimd takes pair 1's first six chains, DVE everything else
including every chain near the tail. The PE (otherwise idle) folds every
partial into the scaled-prefix via one matmul per sub-tile against the
tril*scale constant, accumulating in that pair's PSUM bank, opened by the
F*diag(s) seed and closed by the last sub-tile's matmul. Small sub-tiles
stream last so the post-stream tail is two short fold chains plus the
epilogue (projections -> 64x65 masked softmax, batched over both pairs).
"""

from contextlib import ExitStack

import numpy as np

import concourse.bacc as bacc
import concourse.mybir as mybir
import concourse.tile as tile
from concourse import bass_utils

# Problem constants (hardcoded per contract; kernel.py must be self-contained).
B, HEADS, BUCKETS, DIM, DIM_SORT, T = 4, 8, 64, 64, 8, 8192
BH = B * HEADS            # 32 merged batch*head slices
NCORES = 8
BHC = BH // NCORES        # 4 slices per core
NPAIR = BHC // 2          # 2 pairs per core
CHUNK = T // BUCKETS      # 128 rows per bucket
NEG = -1.0e30             # softmax mask value (underflows exp to exactly 0)
FP = mybir.dt.float32

# packed-constant column offsets: cpack = [lmat_s | s2col], caux packs the
# nonzero rows of the cq/ck seed blocks plus their row-scatter matrix; the
# other structural constants (identity, diag(s), masks) are built on-chip
# by GpSimd affine_selects during its pre-stream idle window.
NC128 = 128 + 1
NC64 = 4 * 104
NCAUX = 2 * 128 + 104
NCALL = NC128

# pair-1 fold chains for sub-tiles [0, GP_CHAINS) run on GpSimd; all other
# chains (including every chain near the tail) on the faster DVE
GP_CHAINS = 6

TRACE = False  # set by test.py for profiling runs
TRACE_KWARGS = {}  # extra run_bass_kernel_spmd kwargs for profiling runs
LAST_RESULTS = None  # BassKernelResults of the most recent run

_PROG_CACHE = {}


def _cascade_sizes(chunk):
    # uniform mid-size tiles, small ones last: (16x7, 8, 4, 4) for chunk=128
    assert chunk == 128, "sub-tile schedule is tuned for chunk=128"
    sizes = [16] * 7 + [8, 4, 4]
    assert sum(sizes) == chunk, (sizes, chunk)
    return sizes


def _build_program(t_seq=T, enable_asserts=False, debug_taps=False):
    chunk = t_seq // BUCKETS
    sizes = _cascade_sizes(chunk)
    nsub = len(sizes)

    nc = bacc.Bacc(
        "TRN2",
        target_bir_lowering=False,
        debug=False,
        enable_asserts=enable_asserts,
        num_devices=NCORES,
    )

    def din(name, shape):
        return nc.dram_tensor(name, shape, FP, kind="ExternalInput").ap()

    kin = din("kin", (BHC, t_seq, DIM))
    # packed constants, three small DMAs:
    # cpack (128, 129)  [lmat_s | s-column]
    # c64   (64, 416)   [wqk_pt_p0 | wqk_pt_p1 | wqk_ft_p0 | wqk_ft_p1]
    # caux  (32, 360)   [c104 packed nonzero rows (256) | row-scatter P (104)]
    cpack = din("cpack", (128, NCALL))
    c64 = din("c64", (64, NC64))
    caux = din("caux", (32, NCAUX))
    rout = nc.dram_tensor(
        "rout", (BHC, BUCKETS, BUCKETS + 1), FP, kind="ExternalOutput"
    ).ap()

    X = mybir.AxisListType.X
    Exp = mybir.ActivationFunctionType.Exp
    MULT = mybir.AluOpType.mult

    with tile.TileContext(nc) as tc:
        with ExitStack() as ctx:
            singles = ctx.enter_context(tc.tile_pool(name="singles", bufs=1))
            kpools = [
                ctx.enter_context(tc.tile_pool(name=f"kpool{s}", bufs=2))
                for s in range(nsub)
            ]
            parp = ctx.enter_context(tc.tile_pool(name="parp", bufs=nsub))
            small = ctx.enter_context(tc.tile_pool(name="small", bufs=2))
            pp = ctx.enter_context(tc.tile_pool(name="pp", bufs=1, space="PSUM"))

            cp_sb = singles.tile([128, NCALL], FP, tag="cpack")
            nc.scalar.dma_start(cp_sb[:], cpack)
            c64_sb = singles.tile([64, NC64], FP, tag="c64")
            nc.scalar.dma_start(c64_sb[:], c64)
            caux_sb = singles.tile([32, NCAUX], FP, tag="caux")
            nc.scalar.dma_start(caux_sb[:], caux)

            # ---- bulk k sub-tile DMAs, single queue, pair 1 leading so
            # its GpSimd chains start first (contiguous rows*256B runs per
            # partition)
            ksrcs = [
                kin[2 * p : 2 * p + 2].rearrange("b (c r) d -> (b c) r d", r=chunk)
                for p in range(NPAIR)
            ]
            kts = {}
            r0 = 0
            for s, rs in enumerate(sizes):
                for p in (1, 0):
                    kt = kpools[s].tile(
                        [128, rs, DIM], FP, tag=f"kt{s}", name=f"kt{s}_{p}"
                    )
                    nc.sync.dma_start(kt[:], ksrcs[p][:, r0 : r0 + rs, :])
                    kts[(p, s)] = kt
                r0 += rs

            lmat_s = cp_sb[:, 0:128]
            s2col = cp_sb[:, 128:129]

            # on-chip structural constants, built by GpSimd (idle until the
            # first bulk tile lands ~11us in) + one Scalar scale op:
            #   eye      128x128 identity
            #   idents   diag(s) (cumavg scales)
            #   am68/mm68: additive causal mask / tril(-1) output mask in the
            #   68-wide-per-pair logit layout (col 0 pad, col 1+j logit j
            #   valid iff j <= row, cols 66:68 pad)
            # neuronxcc implements only is_ge / is_gt / not_equal for
            # affine_select, so every predicate is phrased with those
            GE, GT, NE = (
                mybir.AluOpType.is_ge,
                mybir.AluOpType.is_gt,
                mybir.AluOpType.not_equal,
            )
            eye_sb = singles.tile([128, 128], FP, tag="eye")
            nc.gpsimd.memset(eye_sb[:], 0.0)
            # where(c - i != 0, 0, fill=1) = identity
            nc.gpsimd.affine_select(
                eye_sb[:], eye_sb[:], [[1, 128]], NE, 1.0,
                base=0, channel_multiplier=-1,
            )
            idents_sb = singles.tile([128, 128], FP, tag="idents")
            nc.scalar.activation(
                idents_sb[:], eye_sb[:], mybir.ActivationFunctionType.Copy,
                scale=s2col,
            )
            am_sb = singles.tile([128, 2, 68], FP, tag="am68")
            nc.gpsimd.memset(am_sb[:], 0.0)
            for b in range(2):
                # keep 0 where row - pos + 1 >= 0 (valid logit), else NEG
                nc.gpsimd.affine_select(
                    am_sb[64 * b : 64 * b + 64], am_sb[64 * b : 64 * b + 64],
                    [[0, 2], [-1, 68]], GE, NEG,
                    base=1, channel_multiplier=1,
                )
            # pad columns: keep pos - 1 >= 0, keep 65 - pos >= 0, else NEG
            nc.gpsimd.affine_select(
                am_sb[:], am_sb[:], [[0, 2], [1, 68]], GE, NEG, base=-1,
                channel_multiplier=0,
            )
            nc.gpsimd.affine_select(
                am_sb[:], am_sb[:], [[0, 2], [-1, 68]], GE, NEG, base=65,
                channel_multiplier=0,
            )
            mm_sb = singles.tile([128, 2, 68], FP, tag="mm68")
            nc.gpsimd.memset(mm_sb[:], 1.0)
            for b in range(2):
                # keep 1 where row - pos + 1 > 0 (output tril(-1)), else 0
                nc.gpsimd.affine_select(
                    mm_sb[64 * b : 64 * b + 64], mm_sb[64 * b : 64 * b + 64],
                    [[0, 2], [-1, 68]], GT, 0.0,
                    base=1, channel_multiplier=1,
                )
            nc.gpsimd.affine_select(
                mm_sb[:], mm_sb[:], [[0, 2], [1, 68]], GE, 0.0, base=-1,
                channel_multiplier=0,
            )
            nc.gpsimd.affine_select(
                mm_sb[:], mm_sb[:], [[0, 2], [-1, 68]], GE, 0.0, base=65,
                channel_multiplier=0,
            )
            idents = idents_sb[:]
            ident = eye_sb[:]
            mmask_b = mm_sb[:]

            # ---- PSUM groups, one bank per (pair, tensor): FT_p is F
            # transposed; PT_p is opened by the F*diag(s) seed and closed by
            # that pair's last chunk-sum prefix matmul. F = row 0 of the
            # pair's first bulk tile.
            # PT is ONE bank with rows (pair, d): every par matmul then
            # covers BOTH pairs in a single LDW+MM (lhsT = the whole 128-col
            # par slot), halving PE work and leaving one close on the tail
            PT_ps = pp.tile([128, 128], FP, tag="PT", name="PT_ps")
            FT_ps = [
                pp.tile([64, 128], FP, tag=f"FT{p}", name=f"FT_ps{p}")
                for p in range(NPAIR)
            ]
            for p in range(NPAIR):
                nc.tensor.matmul(
                    FT_ps[p][:],
                    lhsT=kts[(p, 0)][:, 0, :],
                    rhs=ident,
                    start=True,
                    stop=True,
                )
                nc.tensor.matmul(
                    PT_ps[64 * p : 64 * p + 64, :],
                    lhsT=kts[(p, 0)][:, 0, :],
                    rhs=idents,
                    start=True,
                    stop=False,
                    skip_group_check=True,
                )

            # ---- per-sub-tile fold chains + per-sub-tile prefix matmuls.
            # Each (pair, sub-tile) folds independently down to one row (the
            # first fold targets the upper half so row 0 survives in tile 0),
            # writing its own partial-sum slot; the PE folds every partial
            # into the pair's scaled-prefix PSUM bank as it appears.
            pars = [
                parp.tile([128, NPAIR, DIM], FP, tag=f"par{s}", name=f"par{s}")
                for s in range(nsub)
            ]
            for s, rs in enumerate(sizes):
                for p in (1, 0):
                    t = kts[(p, s)]
                    e = nc.gpsimd if (p == 1 and s < GP_CHAINS) else nc.vector
                    h = rs // 2
                    e.tensor_add(t[:, h:rs, :], t[:, h:rs, :], t[:, 0:h, :])
                    lo, xr = h, h
                    while xr > 2:
                        hh = xr // 2
                        e.tensor_add(
                            t[:, lo : lo + hh, :],
                            t[:, lo : lo + hh, :],
                            t[:, lo + hh : lo + xr, :],
                        )
                        xr = hh
                    e.tensor_add(
                        pars[s][:, p, :], t[:, lo, :], t[:, lo + 1, :]
                    )
                nc.tensor.matmul(
                    PT_ps[:],
                    lhsT=pars[s][:].rearrange("q p d -> q (p d)"),
                    rhs=lmat_s,
                    start=False,
                    stop=s == nsub - 1,
                    skip_group_check=True,
                )

            # ---- sort projections (per pair), batched softmax (both pairs)
            PT_sb = [
                small.tile([64, 128], FP, tag=f"PTs{p}", name=f"PT_sb{p}")
                for p in range(NPAIR)
            ]
            FT_sb = [
                small.tile([64, 128], FP, tag=f"FTs{p}", name=f"FT_sb{p}")
                for p in range(NPAIR)
            ]
            for p in range(NPAIR):
                nc.scalar.copy(FT_sb[p][:], FT_ps[p][:])
                # engines can read a PSUM partition offset and write SBUF
                # partition 0, so both pairs' SKQ matmuls stay at base 0
                nc.scalar.copy(PT_sb[p][:], PT_ps[64 * p : 64 * p + 64, :])

            # SKQ rows: 0:40 sort-q blocks (b0 at 0:8, b1 at 32:40),
            #           64:104 sort-k blocks (b0 at 64:72, b1 at 96:104);
            # one PSUM bank per pair: each holds a long-open accumulation group
            # opened by the constant-term matmul (ready at kernel start) and
            # closed by the PT-part matmul (the only one on the critical tail)
            SQs = []
            RKs = []
            for p in range(NPAIR):
                sk_ps_t = pp.tile([104, 128], FP, tag=f"SKQ{p}")
                sk_ps = sk_ps_t[:]
                nc.tensor.matmul(
                    sk_ps,
                    lhsT=caux_sb[:, 256:360],
                    rhs=caux_sb[:, 128 * p : 128 * p + 128],
                    start=True,
                    stop=False,
                    skip_group_check=True,
                )
                nc.tensor.matmul(
                    sk_ps,
                    lhsT=c64_sb[:, 208 + 104 * p : 312 + 104 * p],
                    rhs=FT_sb[p][:],
                    start=False,
                    stop=False,
                    skip_group_check=True,
                )
                nc.tensor.matmul(
                    sk_ps,
                    lhsT=c64_sb[:, 104 * p : 104 * p + 104],
                    rhs=PT_sb[p][:],
                    start=False,
                    stop=True,
                    skip_group_check=True,
                )
                sq_sb = small.tile([40, 128], FP, tag=f"SQ{p}")
                nc.scalar.copy(sq_sb[:], sk_ps[0:40, :])
                rk_sb = small.tile([40, 128], FP, tag=f"RK{p}")
                nc.vector.tensor_copy(rk_sb[:], sk_ps[64:104, :])
                SQs.append(sq_sb)
                RKs.append(rk_sb)

            # R group: opened early by an identity-weighted matmul that seeds
            # the bank with the additive causal mask; the four sq.sk matmuls
            # then accumulate into their quadrants, so the masked logits sit
            # in PSUM with no extra elementwise pass
            # 68-wide per-pair blocks: col 0 pad, col 1 the pad-row's
            # constant zero logit (both from the mask seed), cols 2:66 the
            # sq.sk logits, 66:68 pad; width 68 keeps the partition-64
            # quadrant writes aligned to the PSUM zero-region window
            R_ps = pp.tile([128, 2 * 68], FP, tag="R")
            nc.tensor.matmul(
                R_ps[:],
                lhsT=ident,
                rhs=am_sb[:].rearrange("q p j -> q (p j)"),
                start=True,
                stop=False,
                skip_group_check=True,
            )
            for p in range(NPAIR):
                nc.tensor.matmul(
                    R_ps[0:64, 68 * p + 2 : 68 * p + 66],
                    lhsT=SQs[p][0:8, 0:64],
                    rhs=RKs[p][0:8, 0:64],
                    start=False,
                    stop=False,
                    skip_group_check=True,
                )
                nc.tensor.matmul(
                    R_ps[64:128, 68 * p + 2 : 68 * p + 66],
                    lhsT=SQs[p][32:40, 64:128],
                    rhs=RKs[p][32:40, 64:128],
                    start=False,
                    stop=p == NPAIR - 1,
                    skip_group_check=True,
                )

            # masked softmax over the 65 logits (cols 1:66 of each block;
            # pad cols give exp(NEG) = 0), both pairs batched: cols = (p, j)
            Rm = R_ps[:].rearrange("q (p j) -> q p j", p=2)
            mx = small.tile([128, 2], FP, tag="mx")
            nc.vector.reduce_max(mx[:], Rm, axis=X)
            negm = small.tile([128, 2], FP, tag="negm")
            nc.vector.tensor_scalar(
                negm[:], mx[:], 0.0, -1.0,
                op0=mybir.AluOpType.max, op1=MULT,
            )
            e_sb = small.tile([128, 2, 68], FP, tag="e")
            for p in range(NPAIR):
                nc.scalar.activation(
                    e_sb[:, p, :], R_ps[:, 68 * p : 68 * p + 68], Exp,
                    bias=negm[:, p : p + 1], scale=1.0,
                )
            s1 = small.tile([128, 2], FP, tag="s1")
            nc.vector.reduce_sum(s1[:], e_sb[:], axis=X)
            rin = small.tile([128, 2], FP, tag="rin")
            nc.vector.reciprocal(rin[:], s1[:])
            outt = small.tile([128, 2, 68], FP, tag="outt")
            for p in range(NPAIR):
                # outt = (e * 1/den) * tril-mask, fused
                nc.vector.scalar_tensor_tensor(
                    outt[:, p, :],
                    e_sb[:, p, :],
                    rin[:, p : p + 1],
                    mmask_b[:, p, :],
                    op0=MULT,
                    op1=MULT,
                )
            nc.sync.dma_start(
                rout.rearrange("(p b) i c -> (b i) p c", p=2),
                outt[:, :, 1:66],
            )

    nc.compile()
    return nc


def _get_program(t_seq=T, enable_asserts=False):
    key = (t_seq, enable_asserts)
    if key not in _PROG_CACHE:
        _PROG_CACHE[key] = _build_program(t_seq, enable_asserts=enable_asserts)
    return _PROG_CACHE[key]


def _host_constants(core, q_pos_emb, k_pos_emb, Wsq, Wsk, chunk=CHUNK):
    """Packed per-core constant tensors (two DMAs)."""
    f32 = np.float32
    j = np.arange(64, dtype=np.float64)
    s = (1.0 / (chunk * j + 1.0)).astype(f32)  # per-bucket cumavg scale

    tri = np.triu(np.ones((64, 64), f32), k=1)  # [c, j] = 1 iff c < j
    tri_s = tri * s[None, :]
    lmat_s = np.zeros((128, 128), f32)
    lmat_s[0:64, 0:64] = tri_s
    lmat_s[64:128, 64:128] = tri_s
    idents = np.zeros((128, 128), f32)
    idents[np.arange(128), np.arange(128)] = np.concatenate([s, s])
    ident = np.eye(128, dtype=f32)

    c128 = np.concatenate(
        [lmat_s, np.concatenate([s, s]).reshape(128, 1)], axis=1
    )

    wq_pt = np.zeros((2, 64, 104), f32)   # [pair][d][sq 0:40 | sk 64:104]
    wq_ft = np.zeros((2, 64, 104), f32)
    # c104 packed: only the 4 nonzero 8-row bands (SKQ rows 0:8, 32:40,
    # 64:72, 96:104) as 32 rows, plus the (32 -> 104) row-scatter matrix P
    c104p = np.zeros((2, 32, 128), f32)
    P = np.zeros((32, 104), f32)
    for r in range(32):
        P[r, 32 * (r // 8) + (r % 8)] = 1.0
    for p in range(NPAIR):
        for b in range(2):
            bh = core * BHC + 2 * p + b
            h = bh % HEADS
            r0 = 32 * b
            wq_pt[p, :, r0 : r0 + 8] = Wsq[0, h, 0:64, :]
            wq_pt[p, :, 64 + r0 : 64 + r0 + 8] = Wsk[0, h, 0:64, :]
            wq_ft[p, :, r0 : r0 + 8] = Wsq[0, h, 64:128, :]
            wq_ft[p, :, 64 + r0 : 64 + r0 + 8] = Wsk[0, h, 64:128, :]
            cq = q_pos_emb[0, h] @ Wsq[0, h, 128:192, :]  # (64, 8)
            ck = k_pos_emb[0, h] @ Wsk[0, h, 128:192, :]
            c104p[p, 8 * b : 8 * b + 8, 64 * b : 64 * b + 64] = cq.T
            c104p[p, 16 + 8 * b : 24 + 8 * b, 64 * b : 64 * b + 64] = ck.T

    c64 = np.concatenate([wq_pt[0], wq_pt[1], wq_ft[0], wq_ft[1]], axis=1)
    caux = np.concatenate([c104p[0], c104p[1], P], axis=1)
    assert c128.shape == (128, NCALL), c128.shape
    assert c64.shape == (64, NC64), c64.shape
    assert caux.shape == (32, NCAUX), caux.shape
    return {"cpack": c128, "c64": c64, "caux": caux}


def _run(k, q_pos_emb, k_pos_emb, Wsq, Wsk, trace=False, t_seq=T):
    nc = _get_program(t_seq)
    in_maps = []
    for core in range(NCORES):
        cm = _host_constants(
            core, q_pos_emb, k_pos_emb, Wsq, Wsk, chunk=t_seq // BUCKETS
        )
        cm["kin"] = np.ascontiguousarray(k[core * BHC : (core + 1) * BHC])
        in_maps.append(cm)
    res = bass_utils.run_bass_kernel_spmd(
        nc,
        in_maps,
        core_ids=list(range(NCORES)),
        trace=trace,
        **(TRACE_KWARGS if trace else {}),
    )
    global LAST_RESULTS
    LAST_RESULTS = res
    out = np.concatenate([r["rout"] for r in res.results], axis=0)
    return out, res


def kernel(**inputs):
    k = np.asarray(inputs["k"], np.float32)
    q_pos_emb = np.asarray(inputs["q_pos_emb"], np.float32)
    k_pos_emb = np.asarray(inputs["k_pos_emb"], np.float32)
    Wsq = np.asarray(inputs["Wsq"], np.float32)
    Wsk = np.asarray(inputs["Wsk"], np.float32)
    out, _ = _run(k, q_pos_emb, k_pos_emb, Wsq, Wsk, trace=TRACE)
    return out


# revision 25
# speedup vs baseline: 1.1491x; 1.0124x over previous
"""Trainium2 Bass kernel for CausalAttentionSortNet bucket-scoring.

Math (see reference): only `k` feeds the output. For each merged batch*head
slice, the cumulative-average of k is sampled at bucket starts (every 128th
row), which reduces to per-chunk sums + a strictly-triangular prefix matmul.
The rest is tiny per-bucket sort projections and a 64x65 masked softmax.

Sharding: data-parallel over the merged (batch*heads)=32 axis across 8 cores,
4 slices per core, processed as 2 pairs of 2 slices; a pair fills the
128-partition dim as partition=(slice_in_pair, chunk), free=(row, dim) so
every partition's k data is contiguous 32KB HBM runs (the single-queue bulk
stream saturates all 16 DMA engines at ~350 GB/s).

`q` (half of all input bytes) is never read by the reference computation, so
it is not even transferred to the device.

DMA-instruction budget: the hardware exposes ~12 DMA completion semaphores;
an instruction >=12 positions later reuses an earlier one's semaphore and
its issue blocks until that user completes, so the constants ship in two
early-completing DMAs and every bulk tile is uniform (a version with a slow
small-packet constant DMA in the reuse chain stalled the bulk queue 12us).
Chunk first-rows are not a separate DMA: they arrive inside each pair's
first bulk tile, whose in-place fold targets the tile's upper half so row 0
survives for the F-term matmuls.

Per-chunk reduction: each pair's rows stream as sub-tiles of
(16x7, 8, 4, 4) rows. Mid-stream, SBUF port contention caps DVE at roughly
1.7ns/elem and GpSimd at ~2.9ns/elem (vs 1.04/2.0 idle) and LARGER chains
degrade further (superlinear contention), so each sub-tile gets an
INDEPENDENT halving-fold chain (contiguous tensor_adds down to one row ->
its own partial-sum slot) and the chains are statically balanced across
both engines: GpSimd takes pair 1's first six chains, DVE everything else
including every chain near the tail. The PE (otherwise idle) folds every
partial into the scaled-prefix via one matmul per sub-tile against the
tril*scale constant, accumulating in that pair's PSUM bank, opened by the
F*diag(s) seed and closed by the last sub-tile's matmul. Small sub-tiles
stream last so the post-stream tail is two short fold chains plus the
epilogue (projections -> 64x65 masked softmax, batched over both pairs).
"""

from contextlib import ExitStack

import numpy as np

import concourse.bacc as bacc
import concourse.mybir as mybir
import concourse.tile as tile
from concourse import bass_utils

# Problem constants (hardcoded per contract; kernel.py must be self-contained).
B, HEADS, BUCKETS, DIM, DIM_SORT, T = 4, 8, 64, 64, 8, 8192
BH = B * HEADS            # 32 merged batch*head slices
NCORES = 8
BHC = BH // NCORES        # 4 slices per core
NPAIR = BHC // 2          # 2 pairs per core
CHUNK = T // BUCKETS      # 128 rows per bucket
NEG = -1.0e30             # softmax mask value (underflows exp to exactly 0)
FP = mybir.dt.float32

# packed-constant column offsets: cpack = [lmat_s | s2col], caux packs the
# nonzero rows of the cq/ck seed blocks plus their row-scatter matrix; the
# other structural constants (identity, diag(s), masks) are built on-chip
# by GpSimd affine_selects during its pre-stream idle window.
NC128 = 128 + 1
NC64 = 4 * 104
NCAUX = 2 * 128 + 104
NCALL = NC128

# pair-1 fold chains for sub-tiles [0, GP_CHAINS) run on GpSimd; all other
# chains (including every chain near the tail) on the faster DVE
GP_CHAINS = 8

TRACE = False  # set by test.py for profiling runs
TRACE_KWARGS = {}  # extra run_bass_kernel_spmd kwargs for profiling runs
LAST_RESULTS = None  # BassKernelResults of the most recent run

_PROG_CACHE = {}


def _cascade_sizes(chunk):
    # uniform mid-size tiles, small ones last: (16x7, 8, 4, 4) for chunk=128
    assert chunk == 128, "sub-tile schedule is tuned for chunk=128"
    sizes = [16] * 7 + [8, 4, 4]
    assert sum(sizes) == chunk, (sizes, chunk)
    return sizes


def _build_program(t_seq=T, enable_asserts=False, debug_taps=False):
    chunk = t_seq // BUCKETS
    sizes = _cascade_sizes(chunk)
    nsub = len(sizes)

    nc = bacc.Bacc(
        "TRN2",
        target_bir_lowering=False,
        debug=False,
        enable_asserts=enable_asserts,
        num_devices=NCORES,
    )

    def din(name, shape):
        return nc.dram_tensor(name, shape, FP, kind="ExternalInput").ap()

    kin = din("kin", (BHC, t_seq, DIM))
    # packed constants, three small DMAs:
    # cpack (128, 129)  [lmat_s | s-column]
    # c64   (64, 416)   [wqk_pt_p0 | wqk_pt_p1 | wqk_ft_p0 | wqk_ft_p1]
    # caux  (32, 360)   [c104 packed nonzero rows (256) | row-scatter P (104)]
    cpack = din("cpack", (128, NCALL))
    c64 = din("c64", (64, NC64))
    caux = din("caux", (32, NCAUX))
    rout = nc.dram_tensor(
        "rout", (BHC, BUCKETS, BUCKETS + 1), FP, kind="ExternalOutput"
    ).ap()

    X = mybir.AxisListType.X
    Exp = mybir.ActivationFunctionType.Exp
    MULT = mybir.AluOpType.mult

    with tile.TileContext(nc) as tc:
        with ExitStack() as ctx:
            singles = ctx.enter_context(tc.tile_pool(name="singles", bufs=1))
            kpools = [
                ctx.enter_context(tc.tile_pool(name=f"kpool{s}", bufs=2))
                for s in range(nsub)
            ]
            parp = ctx.enter_context(tc.tile_pool(name="parp", bufs=nsub))
            small = ctx.enter_context(tc.tile_pool(name="small", bufs=2))
            pp = ctx.enter_context(tc.tile_pool(name="pp", bufs=1, space="PSUM"))

            cp_sb = singles.tile([128, NCALL], FP, tag="cpack")
            nc.scalar.dma_start(cp_sb[:], cpack)
            c64_sb = singles.tile([64, NC64], FP, tag="c64")
            nc.scalar.dma_start(c64_sb[:], c64)
            caux_sb = singles.tile([32, NCAUX], FP, tag="caux")
            nc.scalar.dma_start(caux_sb[:], caux)

            # ---- bulk k sub-tile DMAs, single queue, pair 1 leading so
            # its GpSimd chains start first (contiguous rows*256B runs per
            # partition)
            ksrcs = [
                kin[2 * p : 2 * p + 2].rearrange("b (c r) d -> (b c) r d", r=chunk)
                for p in range(NPAIR)
            ]
            kts = {}
            r0 = 0
            for s, rs in enumerate(sizes):
                for p in (1, 0):
                    kt = kpools[s].tile(
                        [128, rs, DIM], FP, tag=f"kt{s}", name=f"kt{s}_{p}"
                    )
                    nc.sync.dma_start(kt[:], ksrcs[p][:, r0 : r0 + rs, :])
                    kts[(p, s)] = kt
                r0 += rs

            lmat_s = cp_sb[:, 0:128]
            s2col = cp_sb[:, 128:129]

            # on-chip structural constants, built by GpSimd (idle until the
            # first bulk tile lands ~11us in) + one Scalar scale op:
            #   eye      128x128 identity
            #   idents   diag(s) (cumavg scales)
            #   am68/mm68: additive causal mask / tril(-1) output mask in the
            #   68-wide-per-pair logit layout (col 0 pad, col 1+j logit j
            #   valid iff j <= row, cols 66:68 pad)
            # neuronxcc implements only is_ge / is_gt / not_equal for
            # affine_select, so every predicate is phrased with those
            GE, GT, NE = (
                mybir.AluOpType.is_ge,
                mybir.AluOpType.is_gt,
                mybir.AluOpType.not_equal,
            )
            eye_sb = singles.tile([128, 128], FP, tag="eye")
            nc.gpsimd.memset(eye_sb[:], 0.0)
            # where(c - i != 0, 0, fill=1) = identity
            nc.gpsimd.affine_select(
                eye_sb[:], eye_sb[:], [[1, 128]], NE, 1.0,
                base=0, channel_multiplier=-1,
            )
            idents_sb = singles.tile([128, 128], FP, tag="idents")
            nc.scalar.activation(
                idents_sb[:], eye_sb[:], mybir.ActivationFunctionType.Copy,
                scale=s2col,
            )
            am_sb = singles.tile([128, 2, 68], FP, tag="am68")
            nc.gpsimd.memset(am_sb[:], 0.0)
            for b in range(2):
                # keep 0 where row - pos + 1 >= 0 (valid logit), else NEG
                nc.gpsimd.affine_select(
                    am_sb[64 * b : 64 * b + 64], am_sb[64 * b : 64 * b + 64],
                    [[0, 2], [-1, 68]], GE, NEG,
                    base=1, channel_multiplier=1,
                )
            # pad columns: keep pos - 1 >= 0, keep 65 - pos >= 0, else NEG
            nc.gpsimd.affine_select(
                am_sb[:], am_sb[:], [[0, 2], [1, 68]], GE, NEG, base=-1,
                channel_multiplier=0,
            )
            nc.gpsimd.affine_select(
                am_sb[:], am_sb[:], [[0, 2], [-1, 68]], GE, NEG, base=65,
                channel_multiplier=0,
            )
            mm_sb = singles.tile([128, 2, 68], FP, tag="mm68")
            nc.gpsimd.memset(mm_sb[:], 1.0)
            for b in range(2):
                # keep 1 where row - pos + 1 > 0 (output tril(-1)), else 0
                nc.gpsimd.affine_select(
                    mm_sb[64 * b : 64 * b + 64], mm_sb[64 * b : 64 * b + 64],
                    [[0, 2], [-1, 68]], GT, 0.0,
                    base=1, channel_multiplier=1,
                )
            nc.gpsimd.affine_select(
                mm_sb[:], mm_sb[:], [[0, 2], [1, 68]], GE, 0.0, base=-1,
                channel_multiplier=0,
            )
            nc.gpsimd.affine_select(
                mm_sb[:], mm_sb[:], [[0, 2], [-1, 68]], GE, 0.0, base=65,
                channel_multiplier=0,
            )
            idents = idents_sb[:]
            ident = eye_sb[:]
            mmask_b = mm_sb[:]

            # ---- PSUM groups, one bank per (pair, tensor): FT_p is F
            # transposed; PT_p is opened by the F*diag(s) seed and closed by
            # that pair's last chunk-sum prefix matmul. F = row 0 of the
            # pair's first bulk tile.
            # PT is ONE bank with rows (pair, d): every par matmul then
            # covers BOTH pairs in a single LDW+MM (lhsT = the whole 128-col
            # par slot), halving PE work and leaving one close on the tail
            PT_ps = pp.tile([128, 128], FP, tag="PT", name="PT_ps")
            FT_ps = [
                pp.tile([64, 128], FP, tag=f"FT{p}", name=f"FT_ps{p}")
                for p in range(NPAIR)
            ]
            for p in range(NPAIR):
                nc.tensor.matmul(
                    FT_ps[p][:],
                    lhsT=kts[(p, 0)][:, 0, :],
                    rhs=ident,
                    start=True,
                    stop=True,
                )
                nc.tensor.matmul(
                    PT_ps[64 * p : 64 * p + 64, :],
                    lhsT=kts[(p, 0)][:, 0, :],
                    rhs=idents,
                    start=True,
                    stop=False,
                    skip_group_check=True,
                )

            # ---- per-sub-tile fold chains + per-sub-tile prefix matmuls.
            # Each (pair, sub-tile) folds independently down to one row (the
            # first fold targets the upper half so row 0 survives in tile 0),
            # writing its own partial-sum slot; the PE folds every partial
            # into the pair's scaled-prefix PSUM bank as it appears.
            pars = [
                parp.tile([128, NPAIR, DIM], FP, tag=f"par{s}", name=f"par{s}")
                for s in range(nsub)
            ]
            for s, rs in enumerate(sizes):
                for p in (1, 0):
                    t = kts[(p, s)]
                    e = nc.gpsimd if (p == 1 and s < GP_CHAINS) else nc.vector
                    h = rs // 2
                    e.tensor_add(t[:, h:rs, :], t[:, h:rs, :], t[:, 0:h, :])
                    lo, xr = h, h
                    while xr > 2:
                        hh = xr // 2
                        e.tensor_add(
                            t[:, lo : lo + hh, :],
                            t[:, lo : lo + hh, :],
                            t[:, lo + hh : lo + xr, :],
                        )
                        xr = hh
                    e.tensor_add(
                        pars[s][:, p, :], t[:, lo, :], t[:, lo + 1, :]
                    )
                nc.tensor.matmul(
                    PT_ps[:],
                    lhsT=pars[s][:].rearrange("q p d -> q (p d)"),
                    rhs=lmat_s,
                    start=False,
                    stop=s == nsub - 1,
                    skip_group_check=True,
                )

            # ---- sort projections (per pair), batched softmax (both pairs)
            PT_sb = [
                small.tile([64, 128], FP, tag=f"PTs{p}", name=f"PT_sb{p}")
                for p in range(NPAIR)
            ]
            FT_sb = [
                small.tile([64, 128], FP, tag=f"FTs{p}", name=f"FT_sb{p}")
                for p in range(NPAIR)
            ]
            for p in range(NPAIR):
                nc.scalar.copy(FT_sb[p][:], FT_ps[p][:])
                # engines can read a PSUM partition offset and write SBUF
                # partition 0, so both pairs' SKQ matmuls stay at base 0
                nc.scalar.copy(PT_sb[p][:], PT_ps[64 * p : 64 * p + 64, :])

            # SKQ rows: 0:40 sort-q blocks (b0 at 0:8, b1 at 32:40),
            #           64:104 sort-k blocks (b0 at 64:72, b1 at 96:104);
            # one PSUM bank per pair: each holds a long-open accumulation group
            # opened by the constant-term matmul (ready at kernel start) and
            # closed by the PT-part matmul (the only one on the critical tail)
            SQs = []
            RKs = []
            for p in range(NPAIR):
                sk_ps_t = pp.tile([104, 128], FP, tag=f"SKQ{p}")
                sk_ps = sk_ps_t[:]
                nc.tensor.matmul(
                    sk_ps,
                    lhsT=caux_sb[:, 256:360],
                    rhs=caux_sb[:, 128 * p : 128 * p + 128],
                    start=True,
                    stop=False,
                    skip_group_check=True,
                )
                nc.tensor.matmul(
                    sk_ps,
                    lhsT=c64_sb[:, 208 + 104 * p : 312 + 104 * p],
                    rhs=FT_sb[p][:],
                    start=False,
                    stop=False,
                    skip_group_check=True,
                )
                nc.tensor.matmul(
                    sk_ps,
                    lhsT=c64_sb[:, 104 * p : 104 * p + 104],
                    rhs=PT_sb[p][:],
                    start=False,
                    stop=True,
                    skip_group_check=True,
                )
                sq_sb = small.tile([40, 128], FP, tag=f"SQ{p}")
                nc.scalar.copy(sq_sb[:], sk_ps[0:40, :])
                rk_sb = small.tile([40, 128], FP, tag=f"RK{p}")
                nc.vector.tensor_copy(rk_sb[:], sk_ps[64:104, :])
                SQs.append(sq_sb)
                RKs.append(rk_sb)

            # R group: opened early by an identity-weighted matmul that seeds
            # the bank with the additive causal mask; the four sq.sk matmuls
            # then accumulate into their quadrants, so the masked logits sit
            # in PSUM with no extra elementwise pass
            # 68-wide per-pair blocks: col 0 pad, col 1 the pad-row's
            # constant zero logit (both from the mask seed), cols 2:66 the
            # sq.sk logits, 66:68 pad; width 68 keeps the partition-64
            # quadrant writes aligned to the PSUM zero-region window
            R_ps = pp.tile([128, 2 * 68], FP, tag="R")
            nc.tensor.matmul(
                R_ps[:],
                lhsT=ident,
                rhs=am_sb[:].rearrange("q p j -> q (p j)"),
                start=True,
                stop=False,
                skip_group_check=True,
            )
            for p in range(NPAIR):
                nc.tensor.matmul(
                    R_ps[0:64, 68 * p + 2 : 68 * p + 66],
                    lhsT=SQs[p][0:8, 0:64],
                    rhs=RKs[p][0:8, 0:64],
                    start=False,
                    stop=False,
                    skip_group_check=True,
                )
                nc.tensor.matmul(
                    R_ps[64:128, 68 * p + 2 : 68 * p + 66],
                    lhsT=SQs[p][32:40, 64:128],
                    rhs=RKs[p][32:40, 64:128],
                    start=False,
                    stop=p == NPAIR - 1,
                    skip_group_check=True,
                )

            # masked softmax over the 65 logits (cols 1:66 of each block;
            # pad cols give exp(NEG) = 0), both pairs batched: cols = (p, j)
            Rm = R_ps[:].rearrange("q (p j) -> q p j", p=2)
            mx = small.tile([128, 2], FP, tag="mx")
            nc.vector.reduce_max(mx[:], Rm, axis=X)
            negm = small.tile([128, 2], FP, tag="negm")
            nc.vector.tensor_scalar(
                negm[:], mx[:], 0.0, -1.0,
                op0=mybir.AluOpType.max, op1=MULT,
            )
            e_sb = small.tile([128, 2, 68], FP, tag="e")
            for p in range(NPAIR):
                nc.scalar.activation(
                    e_sb[:, p, :], R_ps[:, 68 * p : 68 * p + 68], Exp,
                    bias=negm[:, p : p + 1], scale=1.0,
                )
            s1 = small.tile([128, 2], FP, tag="s1")
            nc.vector.reduce_sum(s1[:], e_sb[:], axis=X)
            rin = small.tile([128, 2], FP, tag="rin")
            nc.vector.reciprocal(rin[:], s1[:])
            outt = small.tile([128, 2, 68], FP, tag="outt")
            for p in range(NPAIR):
                # outt = (e * 1/den) * tril-mask, fused
                nc.vector.scalar_tensor_tensor(
                    outt[:, p, :],
                    e_sb[:, p, :],
                    rin[:, p : p + 1],
                    mmask_b[:, p, :],
                    op0=MULT,
                    op1=MULT,
                )
            nc.sync.dma_start(
                rout.rearrange("(p b) i c -> (b i) p c", p=2),
                outt[:, :, 1:66],
            )

    nc.compile()
    return nc


def _get_program(t_seq=T, enable_asserts=False):
    key = (t_seq, enable_asserts)
    if key not in _PROG_CACHE:
        _PROG_CACHE[key] = _build_program(t_seq, enable_asserts=enable_asserts)
    return _PROG_CACHE[key]


def _host_constants(core, q_pos_emb, k_pos_emb, Wsq, Wsk, chunk=CHUNK):
    """Packed per-core constant tensors (two DMAs)."""
    f32 = np.float32
    j = np.arange(64, dtype=np.float64)
    s = (1.0 / (chunk * j + 1.0)).astype(f32)  # per-bucket cumavg scale

    tri = np.triu(np.ones((64, 64), f32), k=1)  # [c, j] = 1 iff c < j
    tri_s = tri * s[None, :]
    lmat_s = np.zeros((128, 128), f32)
    lmat_s[0:64, 0:64] = tri_s
    lmat_s[64:128, 64:128] = tri_s
    idents = np.zeros((128, 128), f32)
    idents[np.arange(128), np.arange(128)] = np.concatenate([s, s])
    ident = np.eye(128, dtype=f32)

    c128 = np.concatenate(
        [lmat_s, np.concatenate([s, s]).reshape(128, 1)], axis=1
    )

    wq_pt = np.zeros((2, 64, 104), f32)   # [pair][d][sq 0:40 | sk 64:104]
    wq_ft = np.zeros((2, 64, 104), f32)
    # c104 packed: only the 4 nonzero 8-row bands (SKQ rows 0:8, 32:40,
    # 64:72, 96:104) as 32 rows, plus the (32 -> 104) row-scatter matrix P
    c104p = np.zeros((2, 32, 128), f32)
    P = np.zeros((32, 104), f32)
    for r in range(32):
        P[r, 32 * (r // 8) + (r % 8)] = 1.0
    for p in range(NPAIR):
        for b in range(2):
            bh = core * BHC + 2 * p + b
            h = bh % HEADS
            r0 = 32 * b
            wq_pt[p, :, r0 : r0 + 8] = Wsq[0, h, 0:64, :]
            wq_pt[p, :, 64 + r0 : 64 + r0 + 8] = Wsk[0, h, 0:64, :]
            wq_ft[p, :, r0 : r0 + 8] = Wsq[0, h, 64:128, :]
            wq_ft[p, :, 64 + r0 : 64 + r0 + 8] = Wsk[0, h, 64:128, :]
            cq = q_pos_emb[0, h] @ Wsq[0, h, 128:192, :]  # (64, 8)
            ck = k_pos_emb[0, h] @ Wsk[0, h, 128:192, :]
            c104p[p, 8 * b : 8 * b + 8, 64 * b : 64 * b + 64] = cq.T
            c104p[p, 16 + 8 * b : 24 + 8 * b, 64 * b : 64 * b + 64] = ck.T

    c64 = np.concatenate([wq_pt[0], wq_pt[1], wq_ft[0], wq_ft[1]], axis=1)
    caux = np.concatenate([c104p[0], c104p[1], P], axis=1)
    assert c128.shape == (128, NCALL), c128.shape
    assert c64.shape == (64, NC64), c64.shape
    assert caux.shape == (32, NCAUX), caux.shape
    return {"cpack": c128, "c64": c64, "caux": caux}


def _run(k, q_pos_emb, k_pos_emb, Wsq, Wsk, trace=False, t_seq=T):
    nc = _get_program(t_seq)
    in_maps = []
    for core in range(NCORES):
        cm = _host_constants(
            core, q_pos_emb, k_pos_emb, Wsq, Wsk, chunk=t_seq // BUCKETS
        )
        cm["kin"] = np.ascontiguousarray(k[core * BHC : (core + 1) * BHC])
        in_maps.append(cm)
    res = bass_utils.run_bass_kernel_spmd(
        nc,
        in_maps,
        core_ids=list(range(NCORES)),
        trace=trace,
        **(TRACE_KWARGS if trace else {}),
    )
    global LAST_RESULTS
    LAST_RESULTS = res
    out = np.concatenate([r["rout"] for r in res.results], axis=0)
    return out, res


def kernel(**inputs):
    k = np.asarray(inputs["k"], np.float32)
    q_pos_emb = np.asarray(inputs["q_pos_emb"], np.float32)
    k_pos_emb = np.asarray(inputs["k_pos_emb"], np.float32)
    Wsq = np.asarray(inputs["Wsq"], np.float32)
    Wsk = np.asarray(inputs["Wsk"], np.float32)
    out, _ = _run(k, q_pos_emb, k_pos_emb, Wsq, Wsk, trace=TRACE)
    return out
